# revision 25
# baseline (speedup 1.0000x reference)
"""Trainium2 Bass kernel for nn_CriticNetwork (3x GATConv + pool + MLP head).

v4 — chunked-collective, two-sweep pass B (sweep-major layouts), window-
grouped edge ops, 8-way graph/data parallel.

- Graphs are contiguous node ranges (batch sorted); core c owns graphs
  [8c, 8c+8) and all edges whose dst lands in its range. Edges sorted by
  (dst window, src chunk, dst); 128-edge tiles never cross a dst window
  nor a src chunk. Chunks split each core's rows at window CBW so the
  h2-row AllGather runs as two overlapping collectives and dma_gather
  indices fit int16.
- Host ships index/permutation data per edge lane: x[src]/x[dst] rows,
  edge_attr rows (window-major for pass A, sweep-major for pass B), rank
  one-hot matrices (oh window-major for pass A; [oh|ohT] sweep-major for
  pass B), int16 gather index tables, the 1/count pooling mask. All
  weight math runs on device.
- Pass A (per group of GA windows): stream oh, build layer-1 + dynamic
  logits (DVE+gpsimd), Prelu+Exp on Act; per window: scatter matmul into
  PSUM, softmax epilogue, node phase (h1 -> elu -> [Ws2|attn-dot] matmul,
  h2 row -> chunk h2slice, dynamic head). Chunk AllGather issued right
  after its last window.
- Pass B sweep k: per group of GB windows: stream [oh|ohT], dma_gather
  h2 rows from H2ext_k (contiguous sweep-major idx), dst attn via ohT
  matmul, msgs = hg*ex; per window: scatter matmul; sweep 0 spills
  partials to SBUF, sweep 1 adds + softmax epilogue + pooling matmul.
  Tiny value-head MLP per core at the end.
"""

import numpy as np
import ml_dtypes

import concourse.bacc as bacc
import concourse.bass as bass
import concourse.mybir as mybir
import concourse.tile as tile
from concourse.masks import make_identity

F32 = mybir.dt.float32
BF16 = mybir.dt.bfloat16
I16 = mybir.dt.int16
AF = mybir.ActivationFunctionType
OP = mybir.AluOpType
NPBF = ml_dtypes.bfloat16

P = 128
H = 4
C = 64
HC = H * C     # 256
HR = 384       # padded H2 row width (bf16) -> 768B
EPS = 1e-16
NEG = 0.2
GCAP = 8       # max tiles per dma_gather call (SWDGE ring: 1024 descriptors)
GA = 3         # pass-A window group
GB = 2         # pass-B window group
CBW = 27       # chunk-0 window count (8*CBW*128 must stay < 2**15)


def brd(ap, pattern, offset=None):
    return bass.AP(ap.tensor, ap.offset if offset is None else offset, pattern)


class Plan:
    pass


# ----------------------------------------------------------------------------
# Host-side sharding / planning (pure index & layout work; no weight math)
# ----------------------------------------------------------------------------

def host_prep(x, edge_attr, edge_index, batch, n_graphs, n_cores):
    N = x.shape[0]
    W = n_cores
    gpc = n_graphs // W
    assert gpc * W == n_graphs

    batch = np.asarray(batch).astype(np.int64)
    src = np.asarray(edge_index[0]).astype(np.int64)
    dst = np.asarray(edge_index[1]).astype(np.int64)
    ea = np.asarray(edge_attr).astype(np.float32)
    x = np.asarray(x).astype(np.float32)

    node_start = np.searchsorted(batch, np.arange(n_graphs + 1))
    core_ns = node_start[0::gpc]
    nk = np.diff(core_ns)
    R = int(128 * np.ceil(nk.max() / 128))
    nwin = R // 128
    cbw = min(CBW, nwin - 1) if nwin > 1 else 1
    CH0 = cbw * 128
    CH1 = R - CH0
    assert W * CH0 < 2 ** 15 and W * CH1 < 2 ** 15

    core_of = np.searchsorted(core_ns, np.arange(N), side="right") - 1
    lid = np.arange(N) - core_ns[core_of]

    counts = np.bincount(batch, minlength=n_graphs).astype(np.float32)
    assert (counts > 0).all()

    dcore = np.searchsorted(core_ns, dst, side="right") - 1
    s_ck = (lid[src] >= CH0).astype(np.int64)
    gidx_all = np.where(
        s_ck == 0, core_of[src] * CH0 + lid[src],
        core_of[src] * CH1 + (lid[src] - CH0))
    assert gidx_all.max() < 2 ** 15

    per_core_sorted = []
    run_len = np.zeros((W, nwin, 2), np.int64)
    for c in range(W):
        m = dcore == c
        dl = (dst[m] - core_ns[c]).astype(np.int64)
        ck = s_ck[m]
        w = dl >> 7
        order = np.lexsort((dl, ck, w))
        e = dict(
            gidx=gidx_all[m][order], dl=dl[order], w=w[order], ck=ck[order],
            ea=ea[m][order], xs=x[src[m]][order], xd=x[dst[m]][order])
        for wi in range(nwin):
            for k in range(2):
                run_len[c, wi, k] = int(np.sum((e["w"] == wi) & (e["ck"] == k)))
        per_core_sorted.append(e)

    tpw = np.maximum(1, np.ceil(run_len.max(axis=0) / P).astype(np.int64))  # [nwin,2]
    T = int(tpw.sum())
    # window-major tile index of (w, k, j): wm_base[w] + (k ? tpw[w,0] : 0) + j
    wm_base = np.concatenate([[0], np.cumsum(tpw.sum(axis=1))]).astype(np.int64)
    # sweep-major tile index of (w, k, j): sm_base[k] + sm_off[k][w] + j
    sm_off = [np.concatenate([[0], np.cumsum(tpw[:, k])]).astype(np.int64)
              for k in range(2)]
    T0 = int(tpw[:, 0].sum())

    per_core_arrays = []
    for c in range(W):
        e = per_core_sorted[c]
        xed = np.zeros((T, P, 8), np.float32)
        eat = np.zeros((T, P, 2), np.float32)
        eatS = np.zeros((T, P, 2), np.float32)
        oh = np.zeros((T, P, P), NPBF)
        ohbS = np.zeros((T, P, 2 * P), NPBF)
        idx16 = np.zeros((T * P,), np.int16)
        run_start = np.zeros((nwin, 2), np.int64)
        e0 = 0
        for wi in range(nwin):
            for k in range(2):
                run_start[wi, k] = e0
                e0 += int(run_len[c, wi, k])
        for wi in range(nwin):
            for k in range(2):
                n_run = int(run_len[c, wi, k])
                e0 = int(run_start[wi, k])
                for j in range(int(tpw[wi, k])):
                    a = e0 + P * j
                    b = min(a + P, e0 + n_run)
                    wm = int(wm_base[wi]) + (int(tpw[wi, 0]) if k else 0) + j
                    sm = (T0 if k else 0) + int(sm_off[k][wi]) + j
                    if b > a:
                        nn = b - a
                        lanes = np.arange(nn)
                        rank = (e["dl"][a:b] - P * wi).astype(np.int64)
                        xed[wm, :nn, 0:4] = e["xs"][a:b]
                        xed[wm, :nn, 4:8] = e["xd"][a:b]
                        eat[wm, :nn] = e["ea"][a:b]
                        oh[wm, lanes, rank] = 1.0
                        eatS[sm, :nn] = e["ea"][a:b]
                        ohbS[sm, lanes, rank] = 1.0
                        ohbS[sm, rank, P + lanes] = 1.0
                        idx16[sm * P:sm * P + nn] = e["gidx"][a:b].astype(np.int16)

        idx_w = idx16.reshape(T * P // 16, 16).T
        idx_rep = np.tile(idx_w, (8, 1))

        ns, ne = int(core_ns[c]), int(core_ns[c + 1])
        pmask = np.zeros((R, gpc), np.float32)
        gix = (batch[ns:ne] - c * gpc).astype(np.int64)
        pmask[np.arange(ne - ns), gix] = 1.0 / counts[batch[ns:ne]]

        per_core_arrays.append(dict(
            xed=np.ascontiguousarray(xed.transpose(1, 0, 2)).astype(NPBF),
            ea_t=np.ascontiguousarray(eat.transpose(1, 0, 2)),
            ea_s=np.ascontiguousarray(eatS.transpose(1, 0, 2)),
            oh=np.ascontiguousarray(oh.transpose(1, 0, 2)),
            ohbS=np.ascontiguousarray(ohbS.transpose(1, 0, 2)),
            idx16=np.ascontiguousarray(idx_rep),
            pmask=pmask.astype(NPBF),
        ))

    plan = Plan()
    plan.W = W
    plan.R = R
    plan.nwin = nwin
    plan.cbw = cbw
    plan.CH = (CH0, CH1)
    plan.T = T
    plan.T0 = T0
    plan.tpw = tpw
    plan.wm_base = wm_base
    plan.sm_off = sm_off
    plan.gpc = gpc
    return plan, per_core_arrays


# ----------------------------------------------------------------------------
# Device program
# ----------------------------------------------------------------------------

def build_bass(plan):
    W, R, nwin, T, T0 = plan.W, plan.R, plan.nwin, plan.T, plan.T0
    CH0, CH1 = plan.CH
    cbw = plan.cbw
    tpw = plan.tpw
    wm_base = plan.wm_base
    sm_off = plan.sm_off
    gpc = plan.gpc

    # group geometry
    ga_groups = [(g, min(g + GA, nwin)) for g in range(0, nwin, GA)]
    GAT = max(int(wm_base[w1] - wm_base[w0]) for w0, w1 in ga_groups)
    gb_groups = [(g, min(g + GB, nwin)) for g in range(0, nwin, GB)]
    GBT = max(max(int(sm_off[k][w1] - sm_off[k][w0]) for w0, w1 in gb_groups)
              for k in range(2))
    KMAX = int(tpw.max())

    nc = bacc.Bacc("TRN2", target_bir_lowering=False, debug=False, num_devices=W)

    def dp(name, shape, dtype=F32, out=False):
        return nc.declare_dram_parameter(name, list(shape), dtype, isOutput=out)

    xed_in = dp("xed", [P, T, 8], BF16)
    ea_in = dp("ea_t", [P, T, 2])
    eaS_in = dp("ea_s", [P, T, 2])
    oh_in = dp("oh", [P, T, P], BF16)
    ohbS_in = dp("ohbS", [P, T, 2 * P], BF16)
    idx_in = dp("idx16", [P, T * 8], I16)
    pmask = dp("pmask", [R, gpc], BF16)

    ws1 = dp("ws1", [1, HC])
    a1s = dp("a1s", [1, HC])
    a1d = dp("a1d", [1, HC])
    we1 = dp("we1", [1, 2 * HC])
    ae1 = dp("ae1", [1, HC])
    bs1 = dp("bs1", [1, HC])
    ws2 = dp("ws2", [HC, HC])
    a2s = dp("a2s", [1, HC])
    a2d = dp("a2d", [1, HC])
    we2 = dp("we2", [1, 2 * HC])
    ae2 = dp("ae2", [1, HC])
    bs2 = dp("bs2", [1, C])
    wd = dp("wd", [3, C])
    wdf = dp("wdf", [1, 3 * C])
    ads = dp("ads", [1, C])
    add_ = dp("add", [1, C])
    bd = dp("bd", [1, C])
    wv1 = dp("wv1", [C, C])
    bv1 = dp("bv1", [1, C])
    wv2 = dp("wv2", [C, 1])
    bv2 = dp("bv2", [1, 1])

    v_out = dp("v", [gpc, 1], out=True)

    h2sl = [nc.dram_tensor("h2slice0", [CH0, HR], BF16),
            nc.dram_tensor("h2slice1", [CH1, HR], BF16)]
    aspace = "Shared" if W > 4 else "Local"
    H2ext = [nc.dram_tensor("H2ext0", [W * CH0, HR], BF16, addr_space=aspace),
             nc.dram_tensor("H2ext1", [W * CH1, HR], BF16, addr_space=aspace)]

    with tile.TileContext(nc) as tc:
        with (
            tc.tile_pool(name="const", bufs=1) as cp,
            tc.tile_pool(name="meta", bufs=1) as mp,
        ):
            ident = cp.tile([P, P], F32)
            make_identity(nc, ident[:])
            ident_bf = cp.tile([P, P], BF16)
            nc.vector.tensor_copy(out=ident_bf[:], in_=ident[:])

            def load_row(dram, width, tag):
                t = cp.tile([1, width], F32, tag=tag)
                nc.sync.dma_start(out=t[:], in_=dram[0:1, 0:width])
                return t

            r_ws1 = load_row(ws1, HC, "r_ws1")
            r_a1s = load_row(a1s, HC, "r_a1s")
            r_a1d = load_row(a1d, HC, "r_a1d")
            r_we1 = load_row(we1, 2 * HC, "r_we1")
            r_ae1 = load_row(ae1, HC, "r_ae1")
            r_bs1 = load_row(bs1, HC, "r_bs1")
            r_a2s = load_row(a2s, HC, "r_a2s")
            r_a2d = load_row(a2d, HC, "r_a2d")
            r_we2 = load_row(we2, 2 * HC, "r_we2")
            r_ae2 = load_row(ae2, HC, "r_ae2")
            r_bs2 = load_row(bs2, C, "r_bs2")
            r_wdf = load_row(wdf, 3 * C, "r_wdf")
            r_ads = load_row(ads, C, "r_ads")
            r_add = load_row(add_, C, "r_add")
            r_bd = load_row(bd, C, "r_bd")
            r_bv1 = load_row(bv1, C, "r_bv1")
            r_bv2 = load_row(bv2, 1, "r_bv2")

            scratch = cp.tile([1, 2 * HC], F32)

            def dot_heads(out_ap, wrow, arow, nh):
                nc.vector.tensor_tensor(
                    out=scratch[0:1, 0:nh * C], in0=wrow, in1=arow, op=OP.mult)
                nc.vector.reduce_sum(
                    out=out_ap,
                    in_=brd(scratch[:], [scratch[:].ap[0], [C, nh], [1, C]]),
                    axis=mybir.AxisListType.X)

            cc_row = cp.tile([1, 2 * H], F32)
            dot_heads(cc_row[0:1, 0:H], r_ws1[:], r_a1s[:], H)
            dot_heads(cc_row[0:1, H:2 * H], r_ws1[:], r_a1d[:], H)
            m_row = cp.tile([1, 4 * H], F32)
            dot_heads(m_row[0:1, 0:H], r_we1[0:1, 0:HC], r_ae1[:], H)
            dot_heads(m_row[0:1, H:2 * H], r_we1[0:1, HC:2 * HC], r_ae1[:], H)
            dot_heads(m_row[0:1, 2 * H:3 * H], r_we2[0:1, 0:HC], r_ae2[:], H)
            dot_heads(m_row[0:1, 3 * H:4 * H], r_we2[0:1, HC:2 * HC], r_ae2[:], H)
            cds_row = cp.tile([1, 6], F32)
            for k, arow in ((0, r_ads), (3, r_add)):
                nc.vector.tensor_tensor(
                    out=brd(scratch[:], [scratch[:].ap[0], [C, 3], [1, C]]),
                    in0=brd(r_wdf[:], [r_wdf[:].ap[0], [C, 3], [1, C]]),
                    in1=brd(arow[:], [arow[:].ap[0], [0, 3], [1, C]]),
                    op=OP.mult)
                nc.vector.reduce_sum(
                    out=cds_row[0:1, k:k + 3],
                    in_=brd(scratch[:], [scratch[:].ap[0], [C, 3], [1, C]]),
                    axis=mybir.AxisListType.X)

            def prep(row_ap, width, tag):
                t = cp.tile([P, width], F32, tag=tag)
                nc.gpsimd.partition_broadcast(t[:], row_ap)
                return t

            cc_rep = prep(cc_row[:], 2 * H, "cc_rep")
            m_rep = prep(m_row[:], 4 * H, "m_rep")
            cds_rep = prep(cds_row[:], 6, "cds_rep")
            w1_rep = prep(r_ws1[:], HC, "w1_rep")
            bs1_rep = prep(r_bs1[:], HC, "bs1_rep")
            a2s_rep = prep(r_a2s[:], HC, "a2s_rep")
            a2d_rep = prep(r_a2d[:], HC, "a2d_rep")
            bs2_rep = prep(r_bs2[:], C, "bs2_rep")
            bd_rep = prep(r_bd[:], C, "bd_rep")
            bv1_rep = prep(r_bv1[:], C, "bv1_rep")
            bv2_rep = prep(r_bv2[:], 1, "bv2_rep")

            ws2_sb = cp.tile([P, 2, HC], F32)
            nc.sync.dma_start(out=ws2_sb[:, 0, :], in_=ws2[0:P, :])
            nc.sync.dma_start(out=ws2_sb[:, 1, :], in_=ws2[P:2 * P, :])
            ws2a_sb = cp.tile([P, 2, HC + 2 * H], BF16)
            nc.vector.tensor_copy(out=ws2a_sb[:, :, 0:HC], in_=ws2_sb[:])
            tmw = cp.tile([P, HC], F32)
            tmr = cp.tile([P, H], F32)
            for ch in range(2):
                for k, arep in ((0, a2s_rep), (H, a2d_rep)):
                    nc.vector.tensor_tensor(
                        out=tmw[:], in0=ws2_sb[:, ch, :], in1=arep[:], op=OP.mult)
                    nc.vector.reduce_sum(
                        out=tmr[:],
                        in_=brd(tmw[:], [tmw[:].ap[0], [C, H], [1, C]]),
                        axis=mybir.AxisListType.X)
                    nc.vector.tensor_copy(
                        out=ws2a_sb[:, ch, HC + k:HC + k + H], in_=tmr[:])

            wd_sb = cp.tile([3, C], BF16)
            wdt = cp.tile([3, C], F32)
            nc.sync.dma_start(out=wdt[:], in_=wd[:])
            nc.vector.tensor_copy(out=wd_sb[:], in_=wdt[:])
            wv1_sb = cp.tile([C, C], F32)
            nc.sync.dma_start(out=wv1_sb[:], in_=wv1[:])
            wv2_sb = cp.tile([C, 1], F32)
            nc.sync.dma_start(out=wv2_sb[:], in_=wv2[:])

            # resident tables
            pm_all = mp.tile([P, nwin, gpc], BF16)
            nc.sync.dma_start(
                out=pm_all[:],
                in_=brd(pmask[:], [[gpc, P], [P * gpc, nwin], [1, gpc]]))

            # alE1 (window-major, layer-1 heads) / alE2 (sweep-major, layer-2)
            alE1 = mp.tile([P, T, 4], BF16)
            alE2 = mp.tile([P, T, 4], BF16)
            from contextlib import ExitStack
            prep_cm = ExitStack()
            pp_prep = prep_cm.enter_context(tc.tile_pool(name="prep", bufs=1))
            ea_sb = pp_prep.tile([P, T, 2], F32)
            nc.sync.dma_start(out=ea_sb[:], in_=ea_in[:])
            eaS_sb = pp_prep.tile([P, T, 2], F32)
            nc.sync.dma_start(out=eaS_sb[:], in_=eaS_in[:])
            tse = pp_prep.tile([P, T], F32)
            for h in range(H):
                nc.vector.tensor_scalar(
                    out=tse[:], in0=ea_sb[:, :, 1],
                    scalar1=m_rep[:, H + h:H + h + 1], scalar2=None, op0=OP.mult)
                nc.vector.scalar_tensor_tensor(
                    out=alE1[:, :, h], in0=ea_sb[:, :, 0],
                    scalar=m_rep[:, h:h + 1], in1=tse[:],
                    op0=OP.mult, op1=OP.add)
            for h in range(H):
                nc.vector.tensor_scalar(
                    out=tse[:], in0=eaS_sb[:, :, 1],
                    scalar1=m_rep[:, 3 * H + h:3 * H + h + 1], scalar2=None,
                    op0=OP.mult)
                nc.vector.scalar_tensor_tensor(
                    out=alE2[:, :, h], in0=eaS_sb[:, :, 0],
                    scalar=m_rep[:, 2 * H + h:2 * H + h + 1], in1=tse[:],
                    op0=OP.mult, op1=OP.add)

            prep_cm.close()
            rA = mp.tile([P, nwin, 2 * H], F32)
            sd2_all = mp.tile([P, nwin, H], BF16)
            hd_sb = mp.tile([P, nwin, C], BF16)
            h_sb = mp.tile([P, nwin, C], BF16)
            part_sb = mp.tile([P, nwin, HC + H], BF16)

            # ---------------- pass A + node phase ------------------------
            with (
                tc.tile_pool(name="ohA", bufs=2) as ohp,
                tc.tile_pool(name="wkA", bufs=2) as wp,
                tc.tile_pool(name="nodeA", bufs=2) as npl,
                tc.tile_pool(name="psA", bufs=2, space="PSUM") as ppa,
                tc.tile_pool(name="psT", bufs=1, space="PSUM") as ppt,
                tc.tile_pool(name="psM", bufs=1, space="PSUM") as ppm,
                tc.tile_pool(name="xedp", bufs=1) as xp,
            ):
                xed_sb = xp.tile([P, T, 8], BF16)
                nc.sync.dma_start(out=xed_sb[:], in_=xed_in[:])
                for w0, w1 in ga_groups:
                    t0 = int(wm_base[w0])
                    t1 = int(wm_base[w1])
                    gt = t1 - t0
                    oh_g = ohp.tile([P, GAT, P], BF16, tag="oh")
                    nc.sync.dma_start(
                        out=oh_g[:, 0:gt, :], in_=oh_in[:, t0:t1, :])

                    al = wp.tile([P, GAT, 5], F32, tag="al")
                    tm4 = wp.tile([P, GAT, 4], F32, tag="tm4")
                    tm6 = wp.tile([P, GAT, 6], F32, tag="tm6")
                    xs = xed_sb[:, t0:t1, :]
                    nc.vector.tensor_tensor(
                        out=al[:, 0:gt, 0:4],
                        in0=brd(cc_rep[:], [cc_rep[:].ap[0], [0, gt], [1, H]]),
                        in1=brd(xs, [xs.ap[0], [8, gt], [0, H]]),
                        op=OP.mult)
                    nc.gpsimd.tensor_tensor(
                        out=tm4[:, 0:gt, :],
                        in0=brd(cc_rep[:], [cc_rep[:].ap[0], [0, gt], [1, H]],
                                offset=cc_rep[:].offset + H),
                        in1=brd(xs, [xs.ap[0], [8, gt], [0, H]],
                                offset=xs.offset + 4),
                        op=OP.mult)
                    nc.vector.tensor_tensor(
                        out=al[:, 0:gt, 0:4], in0=al[:, 0:gt, 0:4],
                        in1=tm4[:, 0:gt, :], op=OP.add)
                    nc.vector.tensor_tensor(
                        out=al[:, 0:gt, 0:4], in0=al[:, 0:gt, 0:4],
                        in1=alE1[:, t0:t1, :], op=OP.add)
                    nc.gpsimd.tensor_tensor(
                        out=tm6[:, 0:gt, :],
                        in0=brd(xs, [xs.ap[0], [8, gt], [4, 2], [1, 3]],
                                offset=xs.offset + 1),
                        in1=brd(cds_rep[:], [cds_rep[:].ap[0], [0, gt], [3, 2], [1, 3]]),
                        op=OP.mult)
                    nc.vector.reduce_sum(
                        out=al[:, 0:gt, 4:5],
                        in_=brd(tm6[:], [tm6[:].ap[0], [6, gt], [1, 6]]),
                        axis=mybir.AxisListType.X)
                    alp = wp.tile([P, GAT, 5], F32, tag="alp")
                    nc.scalar.activation(alp[:, 0:gt, :], al[:, 0:gt, :],
                                         AF.Prelu, alpha=NEG)
                    rhsA = wp.tile([P, GAT, 12], BF16, tag="rhsA")
                    nc.scalar.activation(rhsA[:, 0:gt, 0:5], alp[:, 0:gt, :], AF.Exp)
                    nc.vector.tensor_tensor(
                        out=rhsA[:, 0:gt, 5:9],
                        in0=rhsA[:, 0:gt, 0:4],
                        in1=brd(xs, [xs.ap[0], [8, gt], [0, 4]]),
                        op=OP.mult)
                    nc.vector.tensor_tensor(
                        out=rhsA[:, 0:gt, 9:12],
                        in0=brd(xs, [xs.ap[0], [8, gt], [1, 3]], offset=xs.offset + 1),
                        in1=brd(rhsA[:], [rhsA[:].ap[0], [12, gt], [0, 3]],
                                offset=rhsA[:].offset + 4),
                        op=OP.mult)

                    for w in range(w0, w1):
                        toff = int(wm_base[w]) - t0
                        nt = int(tpw[w, 0] + tpw[w, 1])
                        psA = ppa.tile([P, 12], F32, tag="psA", space="PSUM")
                        for j in range(nt):
                            nc.tensor.matmul(
                                out=psA[:], lhsT=oh_g[:, toff + j, :],
                                rhs=rhsA[:, toff + j, :],
                                start=(j == 0), stop=(j == nt - 1))
                        den = wp.tile([P, 5], F32, tag="den")
                        nc.vector.tensor_scalar(
                            out=den[:], in0=psA[:, 0:5], scalar1=EPS, scalar2=None,
                            op0=OP.add)
                        nc.vector.reciprocal(out=den[:], in_=den[:])
                        nc.vector.tensor_tensor(
                            out=rA[:, w, 0:4], in0=psA[:, 5:9], in1=den[:, 0:4],
                            op=OP.mult)
                        nc.vector.tensor_tensor(
                            out=rA[:, w, 4:7], in0=psA[:, 9:12],
                            in1=den[:, 4:5].to_broadcast([P, 3]), op=OP.mult)

                        h1 = npl.tile([P, HC], F32, tag="h1")
                        nc.vector.tensor_tensor(
                            out=brd(h1[:], [h1[:].ap[0], [C, H], [1, C]]),
                            in0=brd(w1_rep[:], [w1_rep[:].ap[0], [C, H], [1, C]]),
                            in1=brd(rA[:], [rA[:].ap[0], [1, H], [0, C]],
                                    offset=rA[:].offset + w * 2 * H),
                            op=OP.mult)
                        nc.gpsimd.tensor_tensor(
                            out=h1[:], in0=h1[:], in1=bs1_rep[:], op=OP.add)
                        rel = npl.tile([P, HC], F32, tag="rel")
                        nc.scalar.activation(rel[:], h1[:], AF.Relu)
                        nc.gpsimd.tensor_tensor(
                            out=h1[:], in0=h1[:], in1=rel[:], op=OP.subtract)
                        nc.scalar.activation(h1[:], h1[:], AF.Exp)
                        h1e = npl.tile([P, HC], BF16, tag="h1e")
                        nc.vector.scalar_tensor_tensor(
                            out=h1e[:], in0=h1[:], scalar=-1.0, in1=rel[:],
                            op0=OP.add, op1=OP.add)
                        h1t = npl.tile([P, 2, P], BF16, tag="h1t")
                        for ch in range(2):
                            pst = ppt.tile([P, P], BF16, tag="tr", space="PSUM",
                                           bufs=2)
                            nc.tensor.transpose(
                                out=pst[:], in_=h1e[:, ch * P:(ch + 1) * P],
                                identity=ident_bf[:])
                            nc.scalar.copy(out=h1t[:, ch, :], in_=pst[:])
                        ph2 = ppm.tile([P, HC + 2 * H], F32, tag="mm", space="PSUM")
                        for ch in range(2):
                            nc.tensor.matmul(
                                out=ph2[:], lhsT=h1t[:, ch, :], rhs=ws2a_sb[:, ch, :],
                                start=(ch == 0), stop=(ch == 1))
                        h2row = npl.tile([P, HR], BF16, tag="h2row")
                        nc.gpsimd.memset(h2row[:, HC + H:HR], 0.0)
                        nc.scalar.copy(out=h2row[:, 0:HC + H], in_=ph2[:, 0:HC + H])
                        nc.vector.tensor_copy(
                            out=sd2_all[:, w, :], in_=ph2[:, HC + H:HC + 2 * H])
                        if w < cbw:
                            nc.sync.dma_start(
                                out=h2sl[0][w * P:(w + 1) * P, :], in_=h2row[:])
                        else:
                            lw = w - cbw
                            nc.sync.dma_start(
                                out=h2sl[1][lw * P:(lw + 1) * P, :], in_=h2row[:])
                        prd = ppt.tile([P, P], F32, tag="trf", space="PSUM")
                        nc.tensor.transpose(
                            out=prd[0:3, :], in_=rA[:, w, 4:7], identity=ident[:])
                        rdt = npl.tile([3, P], BF16, tag="rdt")
                        nc.vector.tensor_copy(out=rdt[:], in_=prd[0:3, :])
                        phd = ppm.tile([P, C], F32, tag="mmd", space="PSUM")
                        nc.tensor.matmul(
                            out=phd[:], lhsT=rdt[:], rhs=wd_sb[:], start=True,
                            stop=True)
                        nc.vector.tensor_tensor(
                            out=hd_sb[:, w, :], in0=phd[:], in1=bd_rep[:], op=OP.add)
                        if w == cbw - 1 or w == nwin - 1:
                            kc = 0 if w == cbw - 1 else 1
                            nc.gpsimd.collective_compute(
                                "AllGather", OP.bypass,
                                replica_groups=[list(range(W))],
                                ins=[h2sl[kc][:]], outs=[H2ext[kc][:]])

            # ---------------- pass B: two sweeps -------------------------
            with (
                tc.tile_pool(name="ohB", bufs=2) as ohp,
                tc.tile_pool(name="hgB", bufs=2) as hgp,
                tc.tile_pool(name="wkB", bufs=2) as wp,
                tc.tile_pool(name="psB", bufs=2, space="PSUM") as ppb,
                tc.tile_pool(name="psS", bufs=2, space="PSUM") as pps,
                tc.tile_pool(name="psP", bufs=1, space="PSUM") as ppp,
                tc.tile_pool(name="idxp", bufs=1) as ixp,
            ):
                idx_sb = ixp.tile([P, T * 8], I16)
                nc.sync.dma_start(out=idx_sb[:], in_=idx_in[:])
                pg = ppp.tile([gpc, C], F32, tag="pg", space="PSUM")
                for sweep in range(2):
                    sm_base = T0 if sweep else 0
                    for w0, w1 in gb_groups:
                        t0 = sm_base + int(sm_off[sweep][w0])
                        t1 = sm_base + int(sm_off[sweep][w1])
                        gt = t1 - t0
                        ohb_g = ohp.tile([P, GBT, 2 * P], BF16, tag="ohb")
                        nc.sync.dma_start(
                            out=ohb_g[:, 0:gt, :], in_=ohbS_in[:, t0:t1, :])
                        hg = hgp.tile([P, GBT, HR], BF16, tag="hg")
                        for q0 in range(0, gt, GCAP):
                            qn = min(GCAP, gt - q0)
                            ts = t0 + q0
                            nc.gpsimd.dma_gather(
                                out_ap=hg[:, q0:q0 + qn, :],
                                in_ap=H2ext[sweep][:],
                                idxs_ap=idx_sb[:, ts * 8:(ts + qn) * 8],
                                num_idxs=qn * P, num_idxs_reg=qn * P,
                                elem_size=HR)
                        # dst attn values broadcast per window
                        s2dg = pps.tile([P, GBT * H], F32, tag="s2d", space="PSUM")
                        for w in range(w0, w1):
                            toff = int(sm_off[sweep][w]) - int(sm_off[sweep][w0])
                            for j in range(int(tpw[w, sweep])):
                                jj = toff + j
                                nc.tensor.matmul(
                                    out=s2dg[:, jj * H:(jj + 1) * H],
                                    lhsT=ohb_g[:, jj, P:2 * P],
                                    rhs=sd2_all[:, w, :],
                                    start=True, stop=True)
                        al2 = wp.tile([P, GBT, H], F32, tag="al2")
                        nc.vector.tensor_tensor(
                            out=al2[:, 0:gt, :],
                            in0=hg[:, 0:gt, HC:HC + H],
                            in1=brd(s2dg[:], [s2dg[:].ap[0], [H, gt], [1, H]]),
                            op=OP.add)
                        nc.vector.tensor_tensor(
                            out=al2[:, 0:gt, :], in0=al2[:, 0:gt, :],
                            in1=alE2[:, t0:t1, :], op=OP.add)
                        al2p = wp.tile([P, GBT, H], F32, tag="al2p")
                        nc.scalar.activation(al2p[:, 0:gt, :], al2[:, 0:gt, :],
                                             AF.Prelu, alpha=NEG)
                        rhsB = wp.tile([P, GBT, HC + H], BF16, tag="rhsB")
                        nc.scalar.activation(
                            rhsB[:, 0:gt, HC:HC + H], al2p[:, 0:gt, :], AF.Exp)
                        nc.vector.tensor_tensor(
                            out=brd(rhsB[:], [rhsB[:].ap[0], [HC + H, gt], [C, 3], [1, C]]),
                            in0=brd(hg[:], [hg[:].ap[0], [HR, gt], [C, 3], [1, C]]),
                            in1=brd(rhsB[:], [rhsB[:].ap[0], [HC + H, gt], [1, 3], [0, C]],
                                    offset=rhsB[:].offset + HC),
                            op=OP.mult)
                        nc.gpsimd.tensor_tensor(
                            out=brd(rhsB[:], [rhsB[:].ap[0], [HC + H, gt], [1, C]],
                                    offset=rhsB[:].offset + 3 * C),
                            in0=brd(hg[:], [hg[:].ap[0], [HR, gt], [1, C]],
                                    offset=hg[:].offset + 3 * C),
                            in1=brd(rhsB[:], [rhsB[:].ap[0], [HC + H, gt], [0, C]],
                                    offset=rhsB[:].offset + HC + 3),
                            op=OP.mult)
                        for w in range(w0, w1):
                            toff = int(sm_off[sweep][w]) - int(sm_off[sweep][w0])
                            ntk = int(tpw[w, sweep])
                            psB = ppb.tile([P, HC + H], F32, tag="psB", space="PSUM")
                            for j in range(ntk):
                                nc.tensor.matmul(
                                    out=psB[:], lhsT=ohb_g[:, toff + j, 0:P],
                                    rhs=rhsB[:, toff + j, :],
                                    start=(j == 0), stop=(j == ntk - 1))
                            if sweep == 0:
                                nc.vector.tensor_copy(
                                    out=part_sb[:, w, :], in_=psB[:])
                            else:
                                tot = wp.tile([P, HC + H], F32, tag="tot")
                                nc.vector.tensor_tensor(
                                    out=tot[:], in0=psB[:], in1=part_sb[:, w, :],
                                    op=OP.add)
                                dn2 = wp.tile([P, H], F32, tag="dn2")
                                nc.vector.tensor_scalar(
                                    out=dn2[:], in0=tot[:, HC:HC + H], scalar1=EPS,
                                    scalar2=None, op0=OP.add)
                                nc.vector.reciprocal(out=dn2[:], in_=dn2[:])
                                agg = wp.tile([P, HC], F32, tag="agg")
                                nc.vector.tensor_tensor(
                                    out=brd(agg[:], [agg[:].ap[0], [C, H], [1, C]]),
                                    in0=brd(tot[:], [tot[:].ap[0], [C, H], [1, C]]),
                                    in1=brd(dn2[:], [dn2[:].ap[0], [1, H], [0, C]]),
                                    op=OP.mult)
                                hf = wp.tile([P, C], F32, tag="hf")
                                nc.vector.reduce_sum(
                                    out=hf[:],
                                    in_=brd(agg[:], [agg[:].ap[0], [1, C], [C, H]]),
                                    axis=mybir.AxisListType.X)
                                nc.vector.scalar_tensor_tensor(
                                    out=hf[:], in0=hf[:], scalar=0.25,
                                    in1=bs2_rep[:], op0=OP.mult, op1=OP.add)
                                nc.vector.tensor_tensor(
                                    out=h_sb[:, w, :], in0=hf[:],
                                    in1=hd_sb[:, w, :], op=OP.add)
                                nc.tensor.matmul(
                                    out=pg[:], lhsT=pm_all[:, w, :],
                                    rhs=h_sb[:, w, :],
                                    start=(w == 0), stop=(w == nwin - 1))

            # ---------------- value head ---------------------------------
            with (
                tc.tile_pool(name="wkV", bufs=2) as wp,
                tc.tile_pool(name="psV", bufs=2, space="PSUM") as ppv,
            ):
                g_sb = wp.tile([gpc, C], F32, tag="g_sb")
                nc.vector.tensor_copy(out=g_sb[:], in_=pg[:])
                pgt = ppv.tile([C, gpc], F32, tag="tr", space="PSUM")
                nc.tensor.transpose(
                    out=pgt[:], in_=g_sb[:], identity=ident[0:gpc, 0:gpc])
                gt_sb = wp.tile([C, gpc], F32, tag="gt_sb")
                nc.vector.tensor_copy(out=gt_sb[:], in_=pgt[:])
                pv1 = ppv.tile([gpc, C], F32, tag="mm", space="PSUM")
                nc.tensor.matmul(
                    out=pv1[:], lhsT=gt_sb[:], rhs=wv1_sb[:], start=True, stop=True)
                a_sb = wp.tile([gpc, C], F32, tag="a_sb")
                nc.vector.tensor_tensor(
                    out=a_sb[:], in0=pv1[:], in1=bv1_rep[0:gpc, :], op=OP.add)
                nc.vector.tensor_scalar(
                    out=a_sb[:], in0=a_sb[:], scalar1=0.0, scalar2=None, op0=OP.max)
                pat = ppv.tile([C, gpc], F32, tag="tr", space="PSUM")
                nc.tensor.transpose(
                    out=pat[:], in_=a_sb[:], identity=ident[0:gpc, 0:gpc])
                at_sb = wp.tile([C, gpc], F32, tag="at_sb")
                nc.vector.tensor_copy(out=at_sb[:], in_=pat[:])
                pv2 = ppv.tile([gpc, 1], F32, tag="mm2", space="PSUM")
                nc.tensor.matmul(
                    out=pv2[:], lhsT=at_sb[:], rhs=wv2_sb[:], start=True, stop=True)
                vres = wp.tile([gpc, 1], F32, tag="vres")
                nc.vector.tensor_tensor(
                    out=vres[:], in0=pv2[:], in1=bv2_rep[0:gpc, :], op=OP.add)
                nc.sync.dma_start(out=v_out[:], in_=vres[:])

    nc.compile()
    return nc


# ----------------------------------------------------------------------------
# in_maps assembly
# ----------------------------------------------------------------------------

def make_in_maps(plan, per_core_arrays, weights):
    w = {k: np.ascontiguousarray(v, np.float32) for k, v in weights.items()}
    shared = dict(
        ws1=w["Ws1"].reshape(1, HC),
        a1s=w["as_src1"].reshape(1, HC),
        a1d=w["as_dst1"].reshape(1, HC),
        we1=w["We1"].reshape(1, 2 * HC),
        ae1=w["ae1"].reshape(1, HC),
        bs1=w["bs1"].reshape(1, HC),
        ws2=w["Ws2"],
        a2s=w["as_src2"].reshape(1, HC),
        a2d=w["as_dst2"].reshape(1, HC),
        we2=w["We2"].reshape(1, 2 * HC),
        ae2=w["ae2"].reshape(1, HC),
        bs2=w["bs2"].reshape(1, C),
        wd=w["Wd"],
        wdf=w["Wd"].reshape(1, 3 * C),
        ads=w["ad_src"].reshape(1, C),
        add=w["ad_dst"].reshape(1, C),
        bd=w["bd"].reshape(1, C),
        wv1=w["Wv1"],
        bv1=w["bv1"].reshape(1, C),
        wv2=w["Wv2"],
        bv2=w["bv2"].reshape(1, 1),
    )
    in_maps = []
    for c in range(plan.W):
        m = dict(shared)
        m.update(per_core_arrays[c])
        in_maps.append(m)
    return in_maps


_CACHE = {}


def prepare(inputs):
    x = np.asarray(inputs["x"])
    edge_attr = np.asarray(inputs["edge_attr"])
    edge_index = np.asarray(inputs["edge_index"])
    batch = np.asarray(inputs["batch"])
    G = 64
    W = 8
    plan, pca = host_prep(x, edge_attr, edge_index, batch, G, W)
    key = (plan.R, plan.T, plan.cbw, tuple(plan.tpw.ravel()))
    if key not in _CACHE:
        _CACHE[key] = build_bass(plan)
    nc = _CACHE[key]
    weights = {k: inputs[k] for k in (
        "Ws1", "as_src1", "as_dst1", "We1", "ae1", "bs1",
        "Ws2", "as_src2", "as_dst2", "We2", "ae2", "bs2",
        "Wd", "ad_src", "ad_dst", "bd", "Wv1", "bv1", "Wv2", "bv2")}
    in_maps = make_in_maps(plan, pca, weights)
    return nc, in_maps, plan


def kernel(**inputs):
    nc, in_maps, plan = prepare(inputs)
    from concourse.bass_utils import run_bass_kernel_spmd
    res = run_bass_kernel_spmd(nc, in_maps, list(range(plan.W)))
    v = np.concatenate([res.results[c]["v"][:, 0] for c in range(plan.W)])
    return v.astype(np.float32)


# revision 27
# speedup vs baseline: 1.9343x; 1.9343x over previous
"""Trainium2 Bass kernel for nn_CriticNetwork (3x GATConv + pool + MLP head).

v4 — chunked-collective, two-sweep pass B (sweep-major layouts), window-
grouped edge ops, 8-way graph/data parallel.

- Graphs are contiguous node ranges (batch sorted); core c owns graphs
  [8c, 8c+8) and all edges whose dst lands in its range. Edges sorted by
  (dst window, src chunk, dst); 128-edge tiles never cross a dst window
  nor a src chunk. Chunks split each core's rows at window CBW so the
  h2-row AllGather runs as two overlapping collectives and dma_gather
  indices fit int16.
- Host ships index/permutation data per edge lane: x[src]/x[dst] rows,
  edge_attr rows (window-major for pass A, sweep-major for pass B), rank
  one-hot matrices (oh window-major for pass A; [oh|ohT] sweep-major for
  pass B), int16 gather index tables, the 1/count pooling mask. All
  weight math runs on device.
- Pass A (per group of GA windows): stream oh, build layer-1 + dynamic
  logits (DVE+gpsimd), Prelu+Exp on Act; per window: scatter matmul into
  PSUM, softmax epilogue, node phase (h1 -> elu -> [Ws2|attn-dot] matmul,
  h2 row -> chunk h2slice, dynamic head). Chunk AllGather issued right
  after its last window.
- Pass B sweep k: per group of GB windows: stream [oh|ohT], dma_gather
  h2 rows from H2ext_k (contiguous sweep-major idx), dst attn via ohT
  matmul, msgs = hg*ex; per window: scatter matmul; sweep 0 spills
  partials to SBUF, sweep 1 adds + softmax epilogue + pooling matmul.
  Tiny value-head MLP per core at the end.
"""

import numpy as np
import ml_dtypes

import concourse.bacc as bacc
import concourse.bass as bass
import concourse.mybir as mybir
import concourse.tile as tile
from concourse.masks import make_identity

F32 = mybir.dt.float32
BF16 = mybir.dt.bfloat16
I16 = mybir.dt.int16
AF = mybir.ActivationFunctionType
OP = mybir.AluOpType
NPBF = ml_dtypes.bfloat16

P = 128
H = 4
C = 64
HC = H * C     # 256
HR = 384       # padded H2 row width (bf16) -> 768B
EPS = 1e-16
NEG = 0.2
GCAP = 8       # max tiles per dma_gather call (SWDGE ring: 1024 descriptors)
GA = 3         # pass-A window group
GB = 2         # pass-B window group
CBW = 27       # chunk-0 window count (8*CBW*128 must stay < 2**15)


def brd(ap, pattern, offset=None):
    return bass.AP(ap.tensor, ap.offset if offset is None else offset, pattern)


class Plan:
    pass


# ----------------------------------------------------------------------------
# Host-side sharding / planning (pure index & layout work; no weight math)
# ----------------------------------------------------------------------------

def host_prep(x, edge_attr, edge_index, batch, n_graphs, n_cores):
    N = x.shape[0]
    W = n_cores
    gpc = n_graphs // W
    assert gpc * W == n_graphs

    batch = np.asarray(batch).astype(np.int64)
    src = np.asarray(edge_index[0]).astype(np.int64)
    dst = np.asarray(edge_index[1]).astype(np.int64)
    ea = np.asarray(edge_attr).astype(np.float32)
    x = np.asarray(x).astype(np.float32)

    node_start = np.searchsorted(batch, np.arange(n_graphs + 1))
    core_ns = node_start[0::gpc]
    nk = np.diff(core_ns)
    R = int(128 * np.ceil(nk.max() / 128))
    nwin = R // 128
    cbw = min(CBW, nwin - 1) if nwin > 1 else 1
    CH0 = cbw * 128
    CH1 = R - CH0
    assert W * CH0 < 2 ** 15 and W * CH1 < 2 ** 15

    core_of = np.searchsorted(core_ns, np.arange(N), side="right") - 1
    lid = np.arange(N) - core_ns[core_of]

    counts = np.bincount(batch, minlength=n_graphs).astype(np.float32)
    assert (counts > 0).all()

    dcore = np.searchsorted(core_ns, dst, side="right") - 1
    s_ck = (lid[src] >= CH0).astype(np.int64)
    gidx_all = np.where(
        s_ck == 0, core_of[src] * CH0 + lid[src],
        core_of[src] * CH1 + (lid[src] - CH0))
    assert gidx_all.max() < 2 ** 15

    per_core_sorted = []
    run_len = np.zeros((W, nwin, 2), np.int64)
    for c in range(W):
        m = dcore == c
        dl = (dst[m] - core_ns[c]).astype(np.int64)
        ck = s_ck[m]
        w = dl >> 7
        order = np.lexsort((dl, ck, w))
        e = dict(
            gidx=gidx_all[m][order], dl=dl[order], w=w[order], ck=ck[order],
            ea=ea[m][order], xs=x[src[m]][order], xd=x[dst[m]][order])
        for wi in range(nwin):
            for k in range(2):
                run_len[c, wi, k] = int(np.sum((e["w"] == wi) & (e["ck"] == k)))
        per_core_sorted.append(e)

    tpw = np.maximum(1, np.ceil(run_len.max(axis=0) / P).astype(np.int64))  # [nwin,2]
    T = int(tpw.sum())
    # window-major tile index of (w, k, j): wm_base[w] + (k ? tpw[w,0] : 0) + j
    wm_base = np.concatenate([[0], np.cumsum(tpw.sum(axis=1))]).astype(np.int64)
    # sweep-major tile index of (w, k, j): sm_base[k] + sm_off[k][w] + j
    sm_off = [np.concatenate([[0], np.cumsum(tpw[:, k])]).astype(np.int64)
              for k in range(2)]
    T0 = int(tpw[:, 0].sum())

    per_core_arrays = []
    for c in range(W):
        e = per_core_sorted[c]
        xed = np.zeros((T, P, 8), np.float32)
        eat = np.zeros((T, P, 2), np.float32)
        eatS = np.zeros((T, P, 2), np.float32)
        oh = np.zeros((T, P, P), NPBF)
        ohbS = np.zeros((T, P, 2 * P), NPBF)
        idx16 = np.zeros((T * P,), np.int16)
        run_start = np.zeros((nwin, 2), np.int64)
        e0 = 0
        for wi in range(nwin):
            for k in range(2):
                run_start[wi, k] = e0
                e0 += int(run_len[c, wi, k])
        for wi in range(nwin):
            for k in range(2):
                n_run = int(run_len[c, wi, k])
                e0 = int(run_start[wi, k])
                for j in range(int(tpw[wi, k])):
                    a = e0 + P * j
                    b = min(a + P, e0 + n_run)
                    wm = int(wm_base[wi]) + (int(tpw[wi, 0]) if k else 0) + j
                    sm = (T0 if k else 0) + int(sm_off[k][wi]) + j
                    if b > a:
                        nn = b - a
                        lanes = np.arange(nn)
                        rank = (e["dl"][a:b] - P * wi).astype(np.int64)
                        xed[wm, :nn, 0:4] = e["xs"][a:b]
                        xed[wm, :nn, 4:8] = e["xd"][a:b]
                        eat[wm, :nn] = e["ea"][a:b]
                        oh[wm, lanes, rank] = 1.0
                        eatS[sm, :nn] = e["ea"][a:b]
                        ohbS[sm, lanes, rank] = 1.0
                        ohbS[sm, rank, P + lanes] = 1.0
                        idx16[sm * P:sm * P + nn] = e["gidx"][a:b].astype(np.int16)

        idx_w = idx16.reshape(T * P // 16, 16).T
        idx_rep = np.tile(idx_w, (8, 1))

        ns, ne = int(core_ns[c]), int(core_ns[c + 1])
        pmask = np.zeros((R, gpc), np.float32)
        gix = (batch[ns:ne] - c * gpc).astype(np.int64)
        pmask[np.arange(ne - ns), gix] = 1.0 / counts[batch[ns:ne]]

        per_core_arrays.append(dict(
            xed=np.ascontiguousarray(xed.transpose(1, 0, 2)).astype(NPBF),
            ea_t=np.ascontiguousarray(eat.transpose(1, 0, 2)),
            ea_s=np.ascontiguousarray(eatS.transpose(1, 0, 2)),
            oh=np.ascontiguousarray(oh.transpose(1, 0, 2)),
            ohbS=np.ascontiguousarray(ohbS.transpose(1, 0, 2)),
            idx16=np.ascontiguousarray(idx_rep),
            pmask=pmask.astype(NPBF),
        ))

    plan = Plan()
    plan.W = W
    plan.R = R
    plan.nwin = nwin
    plan.cbw = cbw
    plan.CH = (CH0, CH1)
    plan.T = T
    plan.T0 = T0
    plan.tpw = tpw
    plan.wm_base = wm_base
    plan.sm_off = sm_off
    plan.gpc = gpc
    return plan, per_core_arrays


# ----------------------------------------------------------------------------
# Device program
# ----------------------------------------------------------------------------

def build_bass(plan):
    W, R, nwin, T, T0 = plan.W, plan.R, plan.nwin, plan.T, plan.T0
    CH0, CH1 = plan.CH
    cbw = plan.cbw
    tpw = plan.tpw
    wm_base = plan.wm_base
    sm_off = plan.sm_off
    gpc = plan.gpc

    # group geometry
    ga_groups = [(g, min(g + GA, nwin)) for g in range(0, nwin, GA)]
    GAT = max(int(wm_base[w1] - wm_base[w0]) for w0, w1 in ga_groups)
    gb_groups = [(g, min(g + GB, nwin)) for g in range(0, nwin, GB)]
    GBT = max(max(int(sm_off[k][w1] - sm_off[k][w0]) for w0, w1 in gb_groups)
              for k in range(2))
    KMAX = int(tpw.max())

    nc = bacc.Bacc("TRN2", target_bir_lowering=False, debug=False, num_devices=W)

    def dp(name, shape, dtype=F32, out=False):
        return nc.declare_dram_parameter(name, list(shape), dtype, isOutput=out)

    xed_in = dp("xed", [P, T, 8], BF16)
    ea_in = dp("ea_t", [P, T, 2])
    eaS_in = dp("ea_s", [P, T, 2])
    oh_in = dp("oh", [P, T, P], BF16)
    ohbS_in = dp("ohbS", [P, T, 2 * P], BF16)
    idx_in = dp("idx16", [P, T * 8], I16)
    pmask = dp("pmask", [R, gpc], BF16)

    ws1 = dp("ws1", [1, HC])
    a1s = dp("a1s", [1, HC])
    a1d = dp("a1d", [1, HC])
    we1 = dp("we1", [1, 2 * HC])
    ae1 = dp("ae1", [1, HC])
    bs1 = dp("bs1", [1, HC])
    ws2 = dp("ws2", [HC, HC])
    a2s = dp("a2s", [1, HC])
    a2d = dp("a2d", [1, HC])
    we2 = dp("we2", [1, 2 * HC])
    ae2 = dp("ae2", [1, HC])
    bs2 = dp("bs2", [1, C])
    wd = dp("wd", [3, C])
    wdf = dp("wdf", [1, 3 * C])
    ads = dp("ads", [1, C])
    add_ = dp("add", [1, C])
    bd = dp("bd", [1, C])
    wv1 = dp("wv1", [C, C])
    bv1 = dp("bv1", [1, C])
    wv2 = dp("wv2", [C, 1])
    bv2 = dp("bv2", [1, 1])

    v_out = dp("v", [gpc, 1], out=True)

    h2sl = [nc.dram_tensor("h2slice0", [CH0, HR], BF16),
            nc.dram_tensor("h2slice1", [CH1, HR], BF16)]
    aspace = "Shared" if W > 4 else "Local"
    H2ext = [nc.dram_tensor("H2ext0", [W * CH0, HR], BF16, addr_space=aspace),
             nc.dram_tensor("H2ext1", [W * CH1, HR], BF16, addr_space=aspace)]

    with tile.TileContext(nc) as tc:
        with (
            tc.tile_pool(name="const", bufs=1) as cp,
            tc.tile_pool(name="meta", bufs=1) as mp,
        ):
            ident = cp.tile([P, P], F32)
            make_identity(nc, ident[:])
            ident_bf = cp.tile([P, P], BF16)
            nc.vector.tensor_copy(out=ident_bf[:], in_=ident[:])

            def load_row(dram, width, tag):
                t = cp.tile([1, width], F32, tag=tag)
                nc.sync.dma_start(out=t[:], in_=dram[0:1, 0:width])
                return t

            r_ws1 = load_row(ws1, HC, "r_ws1")
            r_a1s = load_row(a1s, HC, "r_a1s")
            r_a1d = load_row(a1d, HC, "r_a1d")
            r_we1 = load_row(we1, 2 * HC, "r_we1")
            r_ae1 = load_row(ae1, HC, "r_ae1")
            r_bs1 = load_row(bs1, HC, "r_bs1")
            r_a2s = load_row(a2s, HC, "r_a2s")
            r_a2d = load_row(a2d, HC, "r_a2d")
            r_we2 = load_row(we2, 2 * HC, "r_we2")
            r_ae2 = load_row(ae2, HC, "r_ae2")
            r_bs2 = load_row(bs2, C, "r_bs2")
            r_wdf = load_row(wdf, 3 * C, "r_wdf")
            r_ads = load_row(ads, C, "r_ads")
            r_add = load_row(add_, C, "r_add")
            r_bd = load_row(bd, C, "r_bd")
            r_bv1 = load_row(bv1, C, "r_bv1")
            r_bv2 = load_row(bv2, 1, "r_bv2")

            scratch = cp.tile([1, 2 * HC], F32)

            def dot_heads(out_ap, wrow, arow, nh):
                nc.vector.tensor_tensor(
                    out=scratch[0:1, 0:nh * C], in0=wrow, in1=arow, op=OP.mult)
                nc.vector.reduce_sum(
                    out=out_ap,
                    in_=brd(scratch[:], [scratch[:].ap[0], [C, nh], [1, C]]),
                    axis=mybir.AxisListType.X)

            cc_row = cp.tile([1, 2 * H], F32)
            dot_heads(cc_row[0:1, 0:H], r_ws1[:], r_a1s[:], H)
            dot_heads(cc_row[0:1, H:2 * H], r_ws1[:], r_a1d[:], H)
            m_row = cp.tile([1, 4 * H], F32)
            dot_heads(m_row[0:1, 0:H], r_we1[0:1, 0:HC], r_ae1[:], H)
            dot_heads(m_row[0:1, H:2 * H], r_we1[0:1, HC:2 * HC], r_ae1[:], H)
            dot_heads(m_row[0:1, 2 * H:3 * H], r_we2[0:1, 0:HC], r_ae2[:], H)
            dot_heads(m_row[0:1, 3 * H:4 * H], r_we2[0:1, HC:2 * HC], r_ae2[:], H)
            cds_row = cp.tile([1, 6], F32)
            for k, arow in ((0, r_ads), (3, r_add)):
                nc.vector.tensor_tensor(
                    out=brd(scratch[:], [scratch[:].ap[0], [C, 3], [1, C]]),
                    in0=brd(r_wdf[:], [r_wdf[:].ap[0], [C, 3], [1, C]]),
                    in1=brd(arow[:], [arow[:].ap[0], [0, 3], [1, C]]),
                    op=OP.mult)
                nc.vector.reduce_sum(
                    out=cds_row[0:1, k:k + 3],
                    in_=brd(scratch[:], [scratch[:].ap[0], [C, 3], [1, C]]),
                    axis=mybir.AxisListType.X)

            def prep(row_ap, width, tag):
                t = cp.tile([P, width], F32, tag=tag)
                nc.gpsimd.partition_broadcast(t[:], row_ap)
                return t

            cc_rep = prep(cc_row[:], 2 * H, "cc_rep")
            m_rep = prep(m_row[:], 4 * H, "m_rep")
            cds_rep = prep(cds_row[:], 6, "cds_rep")
            w1_rep = prep(r_ws1[:], HC, "w1_rep")
            bs1_rep = prep(r_bs1[:], HC, "bs1_rep")
            a2s_rep = prep(r_a2s[:], HC, "a2s_rep")
            a2d_rep = prep(r_a2d[:], HC, "a2d_rep")
            bs2_rep = prep(r_bs2[:], C, "bs2_rep")
            bd_rep = prep(r_bd[:], C, "bd_rep")
            bv1_rep = prep(r_bv1[:], C, "bv1_rep")
            bv2_rep = prep(r_bv2[:], 1, "bv2_rep")

            ws2_sb = cp.tile([P, 2, HC], F32)
            nc.sync.dma_start(out=ws2_sb[:, 0, :], in_=ws2[0:P, :])
            nc.sync.dma_start(out=ws2_sb[:, 1, :], in_=ws2[P:2 * P, :])
            ws2a_sb = cp.tile([P, 2, HC + 2 * H], BF16)
            nc.vector.tensor_copy(out=ws2a_sb[:, :, 0:HC], in_=ws2_sb[:])
            tmw = cp.tile([P, HC], F32)
            tmr = cp.tile([P, H], F32)
            for ch in range(2):
                for k, arep in ((0, a2s_rep), (H, a2d_rep)):
                    nc.vector.tensor_tensor(
                        out=tmw[:], in0=ws2_sb[:, ch, :], in1=arep[:], op=OP.mult)
                    nc.vector.reduce_sum(
                        out=tmr[:],
                        in_=brd(tmw[:], [tmw[:].ap[0], [C, H], [1, C]]),
                        axis=mybir.AxisListType.X)
                    nc.vector.tensor_copy(
                        out=ws2a_sb[:, ch, HC + k:HC + k + H], in_=tmr[:])

            wd_sb = cp.tile([3, C], BF16)
            wdt = cp.tile([3, C], F32)
            nc.sync.dma_start(out=wdt[:], in_=wd[:])
            nc.vector.tensor_copy(out=wd_sb[:], in_=wdt[:])
            wv1_sb = cp.tile([C, C], F32)
            nc.sync.dma_start(out=wv1_sb[:], in_=wv1[:])
            wv2_sb = cp.tile([C, 1], F32)
            nc.sync.dma_start(out=wv2_sb[:], in_=wv2[:])

            # resident tables
            pm_all = mp.tile([P, nwin, gpc], BF16)
            nc.sync.dma_start(
                out=pm_all[:],
                in_=brd(pmask[:], [[gpc, P], [P * gpc, nwin], [1, gpc]]))

            # alE1 (window-major, layer-1 heads) / alE2 (sweep-major, layer-2)
            alE1 = mp.tile([P, T, 4], BF16)
            alE2 = mp.tile([P, T, 4], BF16)
            from contextlib import ExitStack
            prep_cm = ExitStack()
            pp_prep = prep_cm.enter_context(tc.tile_pool(name="prep", bufs=1))
            ea_sb = pp_prep.tile([P, T, 2], F32)
            nc.sync.dma_start(out=ea_sb[:], in_=ea_in[:])
            eaS_sb = pp_prep.tile([P, T, 2], F32)
            nc.sync.dma_start(out=eaS_sb[:], in_=eaS_in[:])
            tse = pp_prep.tile([P, T], F32)
            for h in range(H):
                nc.vector.tensor_scalar(
                    out=tse[:], in0=ea_sb[:, :, 1],
                    scalar1=m_rep[:, H + h:H + h + 1], scalar2=None, op0=OP.mult)
                nc.vector.scalar_tensor_tensor(
                    out=alE1[:, :, h], in0=ea_sb[:, :, 0],
                    scalar=m_rep[:, h:h + 1], in1=tse[:],
                    op0=OP.mult, op1=OP.add)
            for h in range(H):
                nc.vector.tensor_scalar(
                    out=tse[:], in0=eaS_sb[:, :, 1],
                    scalar1=m_rep[:, 3 * H + h:3 * H + h + 1], scalar2=None,
                    op0=OP.mult)
                nc.vector.scalar_tensor_tensor(
                    out=alE2[:, :, h], in0=eaS_sb[:, :, 0],
                    scalar=m_rep[:, 2 * H + h:2 * H + h + 1], in1=tse[:],
                    op0=OP.mult, op1=OP.add)

            prep_cm.close()
            rA = mp.tile([P, nwin, 2 * H], F32)
            sd2_all = mp.tile([P, nwin, H], BF16)
            hd_sb = mp.tile([P, nwin, C], BF16)
            h_sb = mp.tile([P, nwin, C], BF16)
            part_sb = mp.tile([P, nwin, HC + H], BF16)

            # ---------------- pass A + node phase ------------------------
            with (
                tc.tile_pool(name="ohA", bufs=2) as ohp,
                tc.tile_pool(name="wkA", bufs=2) as wp,
                tc.tile_pool(name="nodeA", bufs=2) as npl,
                tc.tile_pool(name="psA", bufs=2, space="PSUM") as ppa,
                tc.tile_pool(name="psT", bufs=1, space="PSUM") as ppt,
                tc.tile_pool(name="psM", bufs=1, space="PSUM") as ppm,
                tc.tile_pool(name="xedp", bufs=1) as xp,
            ):
                xed_sb = xp.tile([P, T, 8], BF16)
                nc.sync.dma_start(out=xed_sb[:], in_=xed_in[:])
                for w0, w1 in ga_groups:
                    t0 = int(wm_base[w0])
                    t1 = int(wm_base[w1])
                    gt = t1 - t0
                    oh_g = ohp.tile([P, GAT, P], BF16, tag="oh")
                    nc.sync.dma_start(
                        out=oh_g[:, 0:gt, :], in_=oh_in[:, t0:t1, :])

                    al = wp.tile([P, GAT, 5], F32, tag="al")
                    tm4 = wp.tile([P, GAT, 4], F32, tag="tm4")
                    tm6 = wp.tile([P, GAT, 6], F32, tag="tm6")
                    xs = xed_sb[:, t0:t1, :]
                    nc.vector.tensor_tensor(
                        out=al[:, 0:gt, 0:4],
                        in0=brd(cc_rep[:], [cc_rep[:].ap[0], [0, gt], [1, H]]),
                        in1=brd(xs, [xs.ap[0], [8, gt], [0, H]]),
                        op=OP.mult)
                    nc.gpsimd.tensor_tensor(
                        out=tm4[:, 0:gt, :],
                        in0=brd(cc_rep[:], [cc_rep[:].ap[0], [0, gt], [1, H]],
                                offset=cc_rep[:].offset + H),
                        in1=brd(xs, [xs.ap[0], [8, gt], [0, H]],
                                offset=xs.offset + 4),
                        op=OP.mult)
                    nc.vector.tensor_tensor(
                        out=al[:, 0:gt, 0:4], in0=al[:, 0:gt, 0:4],
                        in1=tm4[:, 0:gt, :], op=OP.add)
                    nc.vector.tensor_tensor(
                        out=al[:, 0:gt, 0:4], in0=al[:, 0:gt, 0:4],
                        in1=alE1[:, t0:t1, :], op=OP.add)
                    nc.gpsimd.tensor_tensor(
                        out=tm6[:, 0:gt, :],
                        in0=brd(xs, [xs.ap[0], [8, gt], [4, 2], [1, 3]],
                                offset=xs.offset + 1),
                        in1=brd(cds_rep[:], [cds_rep[:].ap[0], [0, gt], [3, 2], [1, 3]]),
                        op=OP.mult)
                    nc.vector.reduce_sum(
                        out=al[:, 0:gt, 4:5],
                        in_=brd(tm6[:], [tm6[:].ap[0], [6, gt], [1, 6]]),
                        axis=mybir.AxisListType.X)
                    alp = wp.tile([P, GAT, 5], F32, tag="alp")
                    nc.scalar.activation(alp[:, 0:gt, :], al[:, 0:gt, :],
                                         AF.Prelu, alpha=NEG)
                    rhsA = wp.tile([P, GAT, 12], BF16, tag="rhsA")
                    nc.scalar.activation(rhsA[:, 0:gt, 0:5], alp[:, 0:gt, :], AF.Exp)
                    nc.vector.tensor_tensor(
                        out=rhsA[:, 0:gt, 5:9],
                        in0=rhsA[:, 0:gt, 0:4],
                        in1=brd(xs, [xs.ap[0], [8, gt], [0, 4]]),
                        op=OP.mult)
                    nc.vector.tensor_tensor(
                        out=rhsA[:, 0:gt, 9:12],
                        in0=brd(xs, [xs.ap[0], [8, gt], [1, 3]], offset=xs.offset + 1),
                        in1=brd(rhsA[:], [rhsA[:].ap[0], [12, gt], [0, 3]],
                                offset=rhsA[:].offset + 4),
                        op=OP.mult)

                    for w in range(w0, w1):
                        toff = int(wm_base[w]) - t0
                        nt = int(tpw[w, 0] + tpw[w, 1])
                        psA = ppa.tile([P, 12], F32, tag="psA", space="PSUM")
                        for j in range(nt):
                            nc.tensor.matmul(
                                out=psA[:], lhsT=oh_g[:, toff + j, :],
                                rhs=rhsA[:, toff + j, :],
                                start=(j == 0), stop=(j == nt - 1))
                        den = wp.tile([P, 5], F32, tag="den")
                        nc.vector.tensor_scalar(
                            out=den[:], in0=psA[:, 0:5], scalar1=EPS, scalar2=None,
                            op0=OP.add)
                        nc.vector.reciprocal(out=den[:], in_=den[:])
                        nc.vector.tensor_tensor(
                            out=rA[:, w, 0:4], in0=psA[:, 5:9], in1=den[:, 0:4],
                            op=OP.mult)
                        nc.vector.tensor_tensor(
                            out=rA[:, w, 4:7], in0=psA[:, 9:12],
                            in1=den[:, 4:5].to_broadcast([P, 3]), op=OP.mult)

                        h1 = npl.tile([P, HC], F32, tag="h1")
                        nc.vector.tensor_tensor(
                            out=brd(h1[:], [h1[:].ap[0], [C, H], [1, C]]),
                            in0=brd(w1_rep[:], [w1_rep[:].ap[0], [C, H], [1, C]]),
                            in1=brd(rA[:], [rA[:].ap[0], [1, H], [0, C]],
                                    offset=rA[:].offset + w * 2 * H),
                            op=OP.mult)
                        nc.gpsimd.tensor_tensor(
                            out=h1[:], in0=h1[:], in1=bs1_rep[:], op=OP.add)
                        rel = npl.tile([P, HC], F32, tag="rel")
                        nc.scalar.activation(rel[:], h1[:], AF.Relu)
                        nc.gpsimd.tensor_tensor(
                            out=h1[:], in0=h1[:], in1=rel[:], op=OP.subtract)
                        nc.scalar.activation(h1[:], h1[:], AF.Exp)
                        h1e = npl.tile([P, HC], BF16, tag="h1e")
                        nc.vector.scalar_tensor_tensor(
                            out=h1e[:], in0=h1[:], scalar=-1.0, in1=rel[:],
                            op0=OP.add, op1=OP.add)
                        h1t = npl.tile([P, 2, P], BF16, tag="h1t")
                        for ch in range(2):
                            pst = ppt.tile([P, P], BF16, tag="tr", space="PSUM",
                                           bufs=2)
                            nc.tensor.transpose(
                                out=pst[:], in_=h1e[:, ch * P:(ch + 1) * P],
                                identity=ident_bf[:])
                            nc.scalar.copy(out=h1t[:, ch, :], in_=pst[:])
                        ph2 = ppm.tile([P, HC + 2 * H], F32, tag="mm", space="PSUM")
                        for ch in range(2):
                            nc.tensor.matmul(
                                out=ph2[:], lhsT=h1t[:, ch, :], rhs=ws2a_sb[:, ch, :],
                                start=(ch == 0), stop=(ch == 1))
                        h2row = npl.tile([P, HR], BF16, tag="h2row")
                        nc.gpsimd.memset(h2row[:, HC + H:HR], 0.0)
                        nc.scalar.copy(out=h2row[:, 0:HC + H], in_=ph2[:, 0:HC + H])
                        nc.vector.tensor_copy(
                            out=sd2_all[:, w, :], in_=ph2[:, HC + H:HC + 2 * H])
                        if w < cbw:
                            nc.sync.dma_start(
                                out=h2sl[0][w * P:(w + 1) * P, :], in_=h2row[:])
                        else:
                            lw = w - cbw
                            nc.sync.dma_start(
                                out=h2sl[1][lw * P:(lw + 1) * P, :], in_=h2row[:])
                        prd = ppt.tile([P, P], F32, tag="trf", space="PSUM")
                        nc.tensor.transpose(
                            out=prd[0:3, :], in_=rA[:, w, 4:7], identity=ident[:])
                        rdt = npl.tile([3, P], BF16, tag="rdt")
                        nc.vector.tensor_copy(out=rdt[:], in_=prd[0:3, :])
                        phd = ppm.tile([P, C], F32, tag="mmd", space="PSUM")
                        nc.tensor.matmul(
                            out=phd[:], lhsT=rdt[:], rhs=wd_sb[:], start=True,
                            stop=True)
                        nc.vector.tensor_tensor(
                            out=hd_sb[:, w, :], in0=phd[:], in1=bd_rep[:], op=OP.add)
                        if w == cbw - 1 or w == nwin - 1:
                            kc = 0 if w == cbw - 1 else 1
                            nc.gpsimd.collective_compute(
                                "AllGather", OP.bypass,
                                replica_groups=[list(range(W))],
                                ins=[h2sl[kc][:]], outs=[H2ext[kc][:]])

            # ---------------- pass B: two sweeps -------------------------
            with (
                tc.tile_pool(name="ohB", bufs=2) as ohp,
                tc.tile_pool(name="hgB", bufs=2) as hgp,
                tc.tile_pool(name="wkB", bufs=2) as wp,
                tc.tile_pool(name="psB", bufs=2, space="PSUM") as ppb,
                tc.tile_pool(name="psS", bufs=2, space="PSUM") as pps,
                tc.tile_pool(name="psP", bufs=1, space="PSUM") as ppp,
                tc.tile_pool(name="idxp", bufs=1) as ixp,
            ):
                idx_sb = ixp.tile([P, T * 8], I16)
                nc.sync.dma_start(out=idx_sb[:], in_=idx_in[:])
                pg = ppp.tile([gpc, C], F32, tag="pg", space="PSUM")
                for sweep in range(2):
                    sm_base = T0 if sweep else 0
                    for w0, w1 in gb_groups:
                        t0 = sm_base + int(sm_off[sweep][w0])
                        t1 = sm_base + int(sm_off[sweep][w1])
                        gt = t1 - t0
                        ohb_g = ohp.tile([P, GBT, 2 * P], BF16, tag="ohb")
                        nc.sync.dma_start(
                            out=ohb_g[:, 0:gt, :], in_=ohbS_in[:, t0:t1, :])
                        hg = hgp.tile([P, GBT, HR], BF16, tag="hg")
                        for q0 in range(0, gt, GCAP):
                            qn = min(GCAP, gt - q0)
                            ts = t0 + q0
                            nc.gpsimd.dma_gather(
                                out_ap=hg[:, q0:q0 + qn, :],
                                in_ap=H2ext[sweep][:],
                                idxs_ap=idx_sb[:, ts * 8:(ts + qn) * 8],
                                num_idxs=qn * P, num_idxs_reg=qn * P,
                                elem_size=HR)
                        # dst attn values broadcast per window
                        s2dg = pps.tile([P, GBT * H], F32, tag="s2d", space="PSUM")
                        for w in range(w0, w1):
                            toff = int(sm_off[sweep][w]) - int(sm_off[sweep][w0])
                            for j in range(int(tpw[w, sweep])):
                                jj = toff + j
                                nc.tensor.matmul(
                                    out=s2dg[:, jj * H:(jj + 1) * H],
                                    lhsT=ohb_g[:, jj, P:2 * P],
                                    rhs=sd2_all[:, w, :],
                                    start=True, stop=True)
                        al2 = wp.tile([P, GBT, H], F32, tag="al2")
                        nc.vector.tensor_tensor(
                            out=al2[:, 0:gt, :],
                            in0=hg[:, 0:gt, HC:HC + H],
                            in1=brd(s2dg[:], [s2dg[:].ap[0], [H, gt], [1, H]]),
                            op=OP.add)
                        nc.vector.tensor_tensor(
                            out=al2[:, 0:gt, :], in0=al2[:, 0:gt, :],
                            in1=alE2[:, t0:t1, :], op=OP.add)
                        al2p = wp.tile([P, GBT, H], F32, tag="al2p")
                        nc.scalar.activation(al2p[:, 0:gt, :], al2[:, 0:gt, :],
                                             AF.Prelu, alpha=NEG)
                        rhsB = wp.tile([P, GBT, HC + H], BF16, tag="rhsB")
                        nc.scalar.activation(
                            rhsB[:, 0:gt, HC:HC + H], al2p[:, 0:gt, :], AF.Exp)
                        nc.vector.tensor_tensor(
                            out=brd(rhsB[:], [rhsB[:].ap[0], [HC + H, gt], [C, 3], [1, C]]),
                            in0=brd(hg[:], [hg[:].ap[0], [HR, gt], [C, 3], [1, C]]),
                            in1=brd(rhsB[:], [rhsB[:].ap[0], [HC + H, gt], [1, 3], [0, C]],
                                    offset=rhsB[:].offset + HC),
                            op=OP.mult)
                        nc.gpsimd.tensor_tensor(
                            out=brd(rhsB[:], [rhsB[:].ap[0], [HC + H, gt], [1, C]],
                                    offset=rhsB[:].offset + 3 * C),
                            in0=brd(hg[:], [hg[:].ap[0], [HR, gt], [1, C]],
                                    offset=hg[:].offset + 3 * C),
                            in1=brd(rhsB[:], [rhsB[:].ap[0], [HC + H, gt], [0, C]],
                                    offset=rhsB[:].offset + HC + 3),
                            op=OP.mult)
                        for w in range(w0, w1):
                            toff = int(sm_off[sweep][w]) - int(sm_off[sweep][w0])
                            ntk = int(tpw[w, sweep])
                            psB = ppb.tile([P, HC + H], F32, tag="psB", space="PSUM")
                            for j in range(ntk):
                                nc.tensor.matmul(
                                    out=psB[:], lhsT=ohb_g[:, toff + j, 0:P],
                                    rhs=rhsB[:, toff + j, :],
                                    start=(j == 0), stop=(j == ntk - 1))
                            if sweep == 0:
                                nc.vector.tensor_copy(
                                    out=part_sb[:, w, :], in_=psB[:])
                            else:
                                tot = wp.tile([P, HC + H], F32, tag="tot")
                                nc.vector.tensor_tensor(
                                    out=tot[:], in0=psB[:], in1=part_sb[:, w, :],
                                    op=OP.add)
                                dn2 = wp.tile([P, H], F32, tag="dn2")
                                nc.vector.tensor_scalar(
                                    out=dn2[:], in0=tot[:, HC:HC + H], scalar1=EPS,
                                    scalar2=None, op0=OP.add)
                                nc.vector.reciprocal(out=dn2[:], in_=dn2[:])
                                agg = wp.tile([P, HC], F32, tag="agg")
                                nc.vector.tensor_tensor(
                                    out=brd(agg[:], [agg[:].ap[0], [C, H], [1, C]]),
                                    in0=brd(tot[:], [tot[:].ap[0], [C, H], [1, C]]),
                                    in1=brd(dn2[:], [dn2[:].ap[0], [1, H], [0, C]]),
                                    op=OP.mult)
                                hf = wp.tile([P, C], F32, tag="hf")
                                nc.vector.reduce_sum(
                                    out=hf[:],
                                    in_=brd(agg[:], [agg[:].ap[0], [1, C], [C, H]]),
                                    axis=mybir.AxisListType.X)
                                nc.vector.scalar_tensor_tensor(
                                    out=hf[:], in0=hf[:], scalar=0.25,
                                    in1=bs2_rep[:], op0=OP.mult, op1=OP.add)
                                nc.vector.tensor_tensor(
                                    out=h_sb[:, w, :], in0=hf[:],
                                    in1=hd_sb[:, w, :], op=OP.add)
                                nc.tensor.matmul(
                                    out=pg[:], lhsT=pm_all[:, w, :],
                                    rhs=h_sb[:, w, :],
                                    start=(w == 0), stop=(w == nwin - 1))

            # ---------------- value head ---------------------------------
            with (
                tc.tile_pool(name="wkV", bufs=2) as wp,
                tc.tile_pool(name="psV", bufs=2, space="PSUM") as ppv,
            ):
                g_sb = wp.tile([gpc, C], F32, tag="g_sb")
                nc.vector.tensor_copy(out=g_sb[:], in_=pg[:])
                pgt = ppv.tile([C, gpc], F32, tag="tr", space="PSUM")
                nc.tensor.transpose(
                    out=pgt[:], in_=g_sb[:], identity=ident[0:gpc, 0:gpc])
                gt_sb = wp.tile([C, gpc], F32, tag="gt_sb")
                nc.vector.tensor_copy(out=gt_sb[:], in_=pgt[:])
                pv1 = ppv.tile([gpc, C], F32, tag="mm", space="PSUM")
                nc.tensor.matmul(
                    out=pv1[:], lhsT=gt_sb[:], rhs=wv1_sb[:], start=True, stop=True)
                a_sb = wp.tile([gpc, C], F32, tag="a_sb")
                nc.vector.tensor_tensor(
                    out=a_sb[:], in0=pv1[:], in1=bv1_rep[0:gpc, :], op=OP.add)
                nc.vector.tensor_scalar(
                    out=a_sb[:], in0=a_sb[:], scalar1=0.0, scalar2=None, op0=OP.max)
                pat = ppv.tile([C, gpc], F32, tag="tr", space="PSUM")
                nc.tensor.transpose(
                    out=pat[:], in_=a_sb[:], identity=ident[0:gpc, 0:gpc])
                at_sb = wp.tile([C, gpc], F32, tag="at_sb")
                nc.vector.tensor_copy(out=at_sb[:], in_=pat[:])
                pv2 = ppv.tile([gpc, 1], F32, tag="mm2", space="PSUM")
                nc.tensor.matmul(
                    out=pv2[:], lhsT=at_sb[:], rhs=wv2_sb[:], start=True, stop=True)
                vres = wp.tile([gpc, 1], F32, tag="vres")
                nc.vector.tensor_tensor(
                    out=vres[:], in0=pv2[:], in1=bv2_rep[0:gpc, :], op=OP.add)
                nc.sync.dma_start(out=v_out[:], in_=vres[:])

    nc.compile()
    return nc


# ----------------------------------------------------------------------------
# in_maps assembly
# ----------------------------------------------------------------------------

def make_in_maps(plan, per_core_arrays, weights):
    w = {k: np.ascontiguousarray(v, np.float32) for k, v in weights.items()}
    shared = dict(
        ws1=w["Ws1"].reshape(1, HC),
        a1s=w["as_src1"].reshape(1, HC),
        a1d=w["as_dst1"].reshape(1, HC),
        we1=w["We1"].reshape(1, 2 * HC),
        ae1=w["ae1"].reshape(1, HC),
        bs1=w["bs1"].reshape(1, HC),
        ws2=w["Ws2"],
        a2s=w["as_src2"].reshape(1, HC),
        a2d=w["as_dst2"].reshape(1, HC),
        we2=w["We2"].reshape(1, 2 * HC),
        ae2=w["ae2"].reshape(1, HC),
        bs2=w["bs2"].reshape(1, C),
        wd=w["Wd"],
        wdf=w["Wd"].reshape(1, 3 * C),
        ads=w["ad_src"].reshape(1, C),
        add=w["ad_dst"].reshape(1, C),
        bd=w["bd"].reshape(1, C),
        wv1=w["Wv1"],
        bv1=w["bv1"].reshape(1, C),
        wv2=w["Wv2"],
        bv2=w["bv2"].reshape(1, 1),
    )
    in_maps = []
    for c in range(plan.W):
        m = dict(shared)
        m.update(per_core_arrays[c])
        in_maps.append(m)
    return in_maps


_CACHE = {}


def prepare(inputs):
    x = np.asarray(inputs["x"])
    edge_attr = np.asarray(inputs["edge_attr"])
    edge_index = np.asarray(inputs["edge_index"])
    batch = np.asarray(inputs["batch"])
    G = 64
    W = 8
    plan, pca = host_prep(x, edge_attr, edge_index, batch, G, W)
    key = (plan.R, plan.T, plan.cbw, tuple(plan.tpw.ravel()))
    if key not in _CACHE:
        _CACHE[key] = build_bass(plan)
    nc = _CACHE[key]
    weights = {k: inputs[k] for k in (
        "Ws1", "as_src1", "as_dst1", "We1", "ae1", "bs1",
        "Ws2", "as_src2", "as_dst2", "We2", "ae2", "bs2",
        "Wd", "ad_src", "ad_dst", "bd", "Wv1", "bv1", "Wv2", "bv2")}
    in_maps = make_in_maps(plan, pca, weights)
    return nc, in_maps, plan


def kernel(**inputs):
    nc, in_maps, plan = prepare(inputs)
    from concourse.bass_utils import run_bass_kernel_spmd
    res = run_bass_kernel_spmd(nc, in_maps, list(range(plan.W)))
    v = np.concatenate([res.results[c]["v"][:, 0] for c in range(plan.W)])
    return v.astype(np.float32)


# revision 28
# speedup vs baseline: 2.8305x; 1.4633x over previous
"""Trainium2 Bass kernel for nn_CriticNetwork (3x GATConv + pool + MLP head).

v4 — chunked-collective, two-sweep pass B (sweep-major layouts), window-
grouped edge ops, 8-way graph/data parallel.

- Graphs are contiguous node ranges (batch sorted); core c owns graphs
  [8c, 8c+8) and all edges whose dst lands in its range. Edges sorted by
  (dst window, src chunk, dst); 128-edge tiles never cross a dst window
  nor a src chunk. Chunks split each core's rows at window CBW so the
  h2-row AllGather runs as two overlapping collectives and dma_gather
  indices fit int16.
- Host ships index/permutation data per edge lane: x[src]/x[dst] rows,
  edge_attr rows (window-major for pass A, sweep-major for pass B), rank
  one-hot matrices (oh window-major for pass A; [oh|ohT] sweep-major for
  pass B), int16 gather index tables, the 1/count pooling mask. All
  weight math runs on device.
- Pass A (per group of GA windows): stream oh, build layer-1 + dynamic
  logits (DVE+gpsimd), Prelu+Exp on Act; per window: scatter matmul into
  PSUM, softmax epilogue, node phase (h1 -> elu -> [Ws2|attn-dot] matmul,
  h2 row -> chunk h2slice, dynamic head). Chunk AllGather issued right
  after its last window.
- Pass B sweep k: per group of GB windows: stream [oh|ohT], dma_gather
  h2 rows from H2ext_k (contiguous sweep-major idx), dst attn via ohT
  matmul, msgs = hg*ex; per window: scatter matmul; sweep 0 spills
  partials to SBUF, sweep 1 adds + softmax epilogue + pooling matmul.
  Tiny value-head MLP per core at the end.
"""

import numpy as np
import ml_dtypes

import concourse.bacc as bacc
import concourse.bass as bass
import concourse.mybir as mybir
import concourse.tile as tile
from concourse.masks import make_identity

F32 = mybir.dt.float32
BF16 = mybir.dt.bfloat16
I16 = mybir.dt.int16
AF = mybir.ActivationFunctionType
OP = mybir.AluOpType
NPBF = ml_dtypes.bfloat16

P = 128
H = 4
C = 64
HC = H * C     # 256
HR = 384       # padded H2 row width (bf16) -> 768B
EPS = 1e-16
NEG = 0.2
GCAP = 8       # max tiles per dma_gather call (SWDGE ring: 1024 descriptors)
GA = 3         # pass-A window group
GB = 2         # pass-B window group
CBW = 27       # chunk-0 window count (8*CBW*128 must stay < 2**15)


def brd(ap, pattern, offset=None):
    return bass.AP(ap.tensor, ap.offset if offset is None else offset, pattern)


class Plan:
    pass


# ----------------------------------------------------------------------------
# Host-side sharding / planning (pure index & layout work; no weight math)
# ----------------------------------------------------------------------------

def host_prep(x, edge_attr, edge_index, batch, n_graphs, n_cores):
    N = x.shape[0]
    W = n_cores
    gpc = n_graphs // W
    assert gpc * W == n_graphs

    batch = np.asarray(batch).astype(np.int64)
    src = np.asarray(edge_index[0]).astype(np.int64)
    dst = np.asarray(edge_index[1]).astype(np.int64)
    ea = np.asarray(edge_attr).astype(np.float32)
    x = np.asarray(x).astype(np.float32)

    node_start = np.searchsorted(batch, np.arange(n_graphs + 1))
    core_ns = node_start[0::gpc]
    nk = np.diff(core_ns)
    R = int(128 * np.ceil(nk.max() / 128))
    nwin = R // 128
    cbw = min(CBW, nwin - 1) if nwin > 1 else 1
    CH0 = cbw * 128
    CH1 = R - CH0
    assert W * CH0 < 2 ** 15 and W * CH1 < 2 ** 15

    core_of = np.searchsorted(core_ns, np.arange(N), side="right") - 1
    lid = np.arange(N) - core_ns[core_of]

    counts = np.bincount(batch, minlength=n_graphs).astype(np.float32)
    assert (counts > 0).all()

    dcore = np.searchsorted(core_ns, dst, side="right") - 1
    s_ck = (lid[src] >= CH0).astype(np.int64)
    gidx_all = np.where(
        s_ck == 0, core_of[src] * CH0 + lid[src],
        core_of[src] * CH1 + (lid[src] - CH0))
    assert gidx_all.max() < 2 ** 15

    per_core_sorted = []
    run_len = np.zeros((W, nwin, 2), np.int64)
    for c in range(W):
        m = dcore == c
        dl = (dst[m] - core_ns[c]).astype(np.int64)
        ck = s_ck[m]
        w = dl >> 7
        order = np.lexsort((dl, ck, w))
        e = dict(
            gidx=gidx_all[m][order], dl=dl[order], w=w[order], ck=ck[order],
            ea=ea[m][order], xs=x[src[m]][order], xd=x[dst[m]][order])
        for wi in range(nwin):
            for k in range(2):
                run_len[c, wi, k] = int(np.sum((e["w"] == wi) & (e["ck"] == k)))
        per_core_sorted.append(e)

    tpw = np.maximum(1, np.ceil(run_len.max(axis=0) / P).astype(np.int64))  # [nwin,2]
    T = int(tpw.sum())
    # window-major tile index of (w, k, j): wm_base[w] + (k ? tpw[w,0] : 0) + j
    wm_base = np.concatenate([[0], np.cumsum(tpw.sum(axis=1))]).astype(np.int64)
    # sweep-major tile index of (w, k, j): sm_base[k] + sm_off[k][w] + j
    sm_off = [np.concatenate([[0], np.cumsum(tpw[:, k])]).astype(np.int64)
              for k in range(2)]
    T0 = int(tpw[:, 0].sum())

    per_core_arrays = []
    for c in range(W):
        e = per_core_sorted[c]
        xed = np.zeros((T, P, 8), np.float32)
        eat = np.zeros((T, P, 2), np.float32)
        eatS = np.zeros((T, P, 2), np.float32)
        oh = np.zeros((T, P, P), NPBF)
        ohbS = np.zeros((T, P, 2 * P), NPBF)
        idx16 = np.zeros((T * P,), np.int16)
        run_start = np.zeros((nwin, 2), np.int64)
        e0 = 0
        for wi in range(nwin):
            for k in range(2):
                run_start[wi, k] = e0
                e0 += int(run_len[c, wi, k])
        for wi in range(nwin):
            for k in range(2):
                n_run = int(run_len[c, wi, k])
                e0 = int(run_start[wi, k])
                for j in range(int(tpw[wi, k])):
                    a = e0 + P * j
                    b = min(a + P, e0 + n_run)
                    wm = int(wm_base[wi]) + (int(tpw[wi, 0]) if k else 0) + j
                    sm = (T0 if k else 0) + int(sm_off[k][wi]) + j
                    if b > a:
                        nn = b - a
                        lanes = np.arange(nn)
                        rank = (e["dl"][a:b] - P * wi).astype(np.int64)
                        xed[wm, :nn, 0:4] = e["xs"][a:b]
                        xed[wm, :nn, 4:8] = e["xd"][a:b]
                        eat[wm, :nn] = e["ea"][a:b]
                        oh[wm, lanes, rank] = 1.0
                        eatS[sm, :nn] = e["ea"][a:b]
                        ohbS[sm, lanes, rank] = 1.0
                        ohbS[sm, rank, P + lanes] = 1.0
                        idx16[sm * P:sm * P + nn] = e["gidx"][a:b].astype(np.int16)

        idx_w = idx16.reshape(T * P // 16, 16).T
        idx_rep = np.tile(idx_w, (8, 1))

        ns, ne = int(core_ns[c]), int(core_ns[c + 1])
        pmask = np.zeros((R, gpc), np.float32)
        gix = (batch[ns:ne] - c * gpc).astype(np.int64)
        pmask[np.arange(ne - ns), gix] = 1.0 / counts[batch[ns:ne]]

        per_core_arrays.append(dict(
            xed=np.ascontiguousarray(xed.transpose(1, 0, 2)).astype(NPBF),
            ea_t=np.ascontiguousarray(eat.transpose(1, 0, 2)),
            ea_s=np.ascontiguousarray(eatS.transpose(1, 0, 2)),
            oh=np.ascontiguousarray(oh.transpose(1, 0, 2)),
            ohbS=np.ascontiguousarray(ohbS.transpose(1, 0, 2)),
            idx16=np.ascontiguousarray(idx_rep),
            pmask=pmask.astype(NPBF),
        ))

    plan = Plan()
    plan.W = W
    plan.R = R
    plan.nwin = nwin
    plan.cbw = cbw
    plan.CH = (CH0, CH1)
    plan.T = T
    plan.T0 = T0
    plan.tpw = tpw
    plan.wm_base = wm_base
    plan.sm_off = sm_off
    plan.gpc = gpc
    return plan, per_core_arrays


# ----------------------------------------------------------------------------
# Device program
# ----------------------------------------------------------------------------

def build_bass(plan):
    W, R, nwin, T, T0 = plan.W, plan.R, plan.nwin, plan.T, plan.T0
    CH0, CH1 = plan.CH
    cbw = plan.cbw
    tpw = plan.tpw
    wm_base = plan.wm_base
    sm_off = plan.sm_off
    gpc = plan.gpc

    # group geometry
    ga_groups = [(g, min(g + GA, nwin)) for g in range(0, nwin, GA)]
    GAT = max(int(wm_base[w1] - wm_base[w0]) for w0, w1 in ga_groups)
    gb_groups = [(g, min(g + GB, nwin)) for g in range(0, nwin, GB)]
    GBT = max(max(int(sm_off[k][w1] - sm_off[k][w0]) for w0, w1 in gb_groups)
              for k in range(2))
    KMAX = int(tpw.max())

    nc = bacc.Bacc("TRN2", target_bir_lowering=False, debug=False, num_devices=W)

    def dp(name, shape, dtype=F32, out=False):
        return nc.declare_dram_parameter(name, list(shape), dtype, isOutput=out)

    xed_in = dp("xed", [P, T, 8], BF16)
    ea_in = dp("ea_t", [P, T, 2])
    eaS_in = dp("ea_s", [P, T, 2])
    oh_in = dp("oh", [P, T, P], BF16)
    ohbS_in = dp("ohbS", [P, T, 2 * P], BF16)
    idx_in = dp("idx16", [P, T * 8], I16)
    pmask = dp("pmask", [R, gpc], BF16)

    ws1 = dp("ws1", [1, HC])
    a1s = dp("a1s", [1, HC])
    a1d = dp("a1d", [1, HC])
    we1 = dp("we1", [1, 2 * HC])
    ae1 = dp("ae1", [1, HC])
    bs1 = dp("bs1", [1, HC])
    ws2 = dp("ws2", [HC, HC])
    a2s = dp("a2s", [1, HC])
    a2d = dp("a2d", [1, HC])
    we2 = dp("we2", [1, 2 * HC])
    ae2 = dp("ae2", [1, HC])
    bs2 = dp("bs2", [1, C])
    wd = dp("wd", [3, C])
    wdf = dp("wdf", [1, 3 * C])
    ads = dp("ads", [1, C])
    add_ = dp("add", [1, C])
    bd = dp("bd", [1, C])
    wv1 = dp("wv1", [C, C])
    bv1 = dp("bv1", [1, C])
    wv2 = dp("wv2", [C, 1])
    bv2 = dp("bv2", [1, 1])

    v_out = dp("v", [gpc, 1], out=True)

    h2sl = [nc.dram_tensor("h2slice0", [CH0, HR], BF16),
            nc.dram_tensor("h2slice1", [CH1, HR], BF16)]
    aspace = "Shared" if W > 4 else "Local"
    H2ext = [nc.dram_tensor("H2ext0", [W * CH0, HR], BF16, addr_space=aspace),
             nc.dram_tensor("H2ext1", [W * CH1, HR], BF16, addr_space=aspace)]

    with tile.TileContext(nc) as tc:
        with (
            tc.tile_pool(name="const", bufs=1) as cp,
            tc.tile_pool(name="meta", bufs=1) as mp,
        ):
            ident = cp.tile([P, P], F32)
            make_identity(nc, ident[:])
            ident_bf = cp.tile([P, P], BF16)
            nc.vector.tensor_copy(out=ident_bf[:], in_=ident[:])

            def load_row(dram, width, tag):
                t = cp.tile([1, width], F32, tag=tag)
                nc.sync.dma_start(out=t[:], in_=dram[0:1, 0:width])
                return t

            r_ws1 = load_row(ws1, HC, "r_ws1")
            r_a1s = load_row(a1s, HC, "r_a1s")
            r_a1d = load_row(a1d, HC, "r_a1d")
            r_we1 = load_row(we1, 2 * HC, "r_we1")
            r_ae1 = load_row(ae1, HC, "r_ae1")
            r_bs1 = load_row(bs1, HC, "r_bs1")
            r_a2s = load_row(a2s, HC, "r_a2s")
            r_a2d = load_row(a2d, HC, "r_a2d")
            r_we2 = load_row(we2, 2 * HC, "r_we2")
            r_ae2 = load_row(ae2, HC, "r_ae2")
            r_bs2 = load_row(bs2, C, "r_bs2")
            r_wdf = load_row(wdf, 3 * C, "r_wdf")
            r_ads = load_row(ads, C, "r_ads")
            r_add = load_row(add_, C, "r_add")
            r_bd = load_row(bd, C, "r_bd")
            r_bv1 = load_row(bv1, C, "r_bv1")
            r_bv2 = load_row(bv2, 1, "r_bv2")

            scratch = cp.tile([1, 2 * HC], F32)

            def dot_heads(out_ap, wrow, arow, nh):
                nc.vector.tensor_tensor(
                    out=scratch[0:1, 0:nh * C], in0=wrow, in1=arow, op=OP.mult)
                nc.vector.reduce_sum(
                    out=out_ap,
                    in_=brd(scratch[:], [scratch[:].ap[0], [C, nh], [1, C]]),
                    axis=mybir.AxisListType.X)

            cc_row = cp.tile([1, 2 * H], F32)
            dot_heads(cc_row[0:1, 0:H], r_ws1[:], r_a1s[:], H)
            dot_heads(cc_row[0:1, H:2 * H], r_ws1[:], r_a1d[:], H)
            m_row = cp.tile([1, 4 * H], F32)
            dot_heads(m_row[0:1, 0:H], r_we1[0:1, 0:HC], r_ae1[:], H)
            dot_heads(m_row[0:1, H:2 * H], r_we1[0:1, HC:2 * HC], r_ae1[:], H)
            dot_heads(m_row[0:1, 2 * H:3 * H], r_we2[0:1, 0:HC], r_ae2[:], H)
            dot_heads(m_row[0:1, 3 * H:4 * H], r_we2[0:1, HC:2 * HC], r_ae2[:], H)
            cds_row = cp.tile([1, 6], F32)
            for k, arow in ((0, r_ads), (3, r_add)):
                nc.vector.tensor_tensor(
                    out=brd(scratch[:], [scratch[:].ap[0], [C, 3], [1, C]]),
                    in0=brd(r_wdf[:], [r_wdf[:].ap[0], [C, 3], [1, C]]),
                    in1=brd(arow[:], [arow[:].ap[0], [0, 3], [1, C]]),
                    op=OP.mult)
                nc.vector.reduce_sum(
                    out=cds_row[0:1, k:k + 3],
                    in_=brd(scratch[:], [scratch[:].ap[0], [C, 3], [1, C]]),
                    axis=mybir.AxisListType.X)

            def prep(row_ap, width, tag):
                t = cp.tile([P, width], F32, tag=tag)
                nc.gpsimd.partition_broadcast(t[:], row_ap)
                return t

            cc_rep = prep(cc_row[:], 2 * H, "cc_rep")
            m_rep = prep(m_row[:], 4 * H, "m_rep")
            cds_rep = prep(cds_row[:], 6, "cds_rep")
            w1_rep = prep(r_ws1[:], HC, "w1_rep")
            bs1_rep = prep(r_bs1[:], HC, "bs1_rep")
            a2s_rep = prep(r_a2s[:], HC, "a2s_rep")
            a2d_rep = prep(r_a2d[:], HC, "a2d_rep")
            bs2_rep = prep(r_bs2[:], C, "bs2_rep")
            bd_rep = prep(r_bd[:], C, "bd_rep")
            bv1_rep = prep(r_bv1[:], C, "bv1_rep")
            bv2_rep = prep(r_bv2[:], 1, "bv2_rep")

            ws2_sb = cp.tile([P, 2, HC], F32)
            nc.sync.dma_start(out=ws2_sb[:, 0, :], in_=ws2[0:P, :])
            nc.sync.dma_start(out=ws2_sb[:, 1, :], in_=ws2[P:2 * P, :])
            ws2a_sb = cp.tile([P, 2, HC + 2 * H], BF16)
            nc.vector.tensor_copy(out=ws2a_sb[:, :, 0:HC], in_=ws2_sb[:])
            tmw = cp.tile([P, HC], F32)
            tmr = cp.tile([P, H], F32)
            for ch in range(2):
                for k, arep in ((0, a2s_rep), (H, a2d_rep)):
                    nc.vector.tensor_tensor(
                        out=tmw[:], in0=ws2_sb[:, ch, :], in1=arep[:], op=OP.mult)
                    nc.vector.reduce_sum(
                        out=tmr[:],
                        in_=brd(tmw[:], [tmw[:].ap[0], [C, H], [1, C]]),
                        axis=mybir.AxisListType.X)
                    nc.vector.tensor_copy(
                        out=ws2a_sb[:, ch, HC + k:HC + k + H], in_=tmr[:])

            wd_sb = cp.tile([3, C], BF16)
            wdt = cp.tile([3, C], F32)
            nc.sync.dma_start(out=wdt[:], in_=wd[:])
            nc.vector.tensor_copy(out=wd_sb[:], in_=wdt[:])
            wv1_sb = cp.tile([C, C], F32)
            nc.sync.dma_start(out=wv1_sb[:], in_=wv1[:])
            wv2_sb = cp.tile([C, 1], F32)
            nc.sync.dma_start(out=wv2_sb[:], in_=wv2[:])

            # resident tables
            pm_all = mp.tile([P, nwin, gpc], BF16)
            nc.sync.dma_start(
                out=pm_all[:],
                in_=brd(pmask[:], [[gpc, P], [P * gpc, nwin], [1, gpc]]))

            # alE1 (window-major, layer-1 heads) / alE2 (sweep-major, layer-2)
            alE1 = mp.tile([P, T, 4], BF16)
            alE2 = mp.tile([P, T, 4], BF16)
            from contextlib import ExitStack
            prep_cm = ExitStack()
            pp_prep = prep_cm.enter_context(tc.tile_pool(name="prep", bufs=1))
            ea_sb = pp_prep.tile([P, T, 2], F32)
            nc.sync.dma_start(out=ea_sb[:], in_=ea_in[:])
            eaS_sb = pp_prep.tile([P, T, 2], F32)
            nc.sync.dma_start(out=eaS_sb[:], in_=eaS_in[:])
            tse = pp_prep.tile([P, T], F32)
            for h in range(H):
                nc.vector.tensor_scalar(
                    out=tse[:], in0=ea_sb[:, :, 1],
                    scalar1=m_rep[:, H + h:H + h + 1], scalar2=None, op0=OP.mult)
                nc.vector.scalar_tensor_tensor(
                    out=alE1[:, :, h], in0=ea_sb[:, :, 0],
                    scalar=m_rep[:, h:h + 1], in1=tse[:],
                    op0=OP.mult, op1=OP.add)
            for h in range(H):
                nc.vector.tensor_scalar(
                    out=tse[:], in0=eaS_sb[:, :, 1],
                    scalar1=m_rep[:, 3 * H + h:3 * H + h + 1], scalar2=None,
                    op0=OP.mult)
                nc.vector.scalar_tensor_tensor(
                    out=alE2[:, :, h], in0=eaS_sb[:, :, 0],
                    scalar=m_rep[:, 2 * H + h:2 * H + h + 1], in1=tse[:],
                    op0=OP.mult, op1=OP.add)

            prep_cm.close()
            rA = mp.tile([P, nwin, 2 * H], F32)
            sd2_all = mp.tile([P, nwin, H], BF16)
            hd_sb = mp.tile([P, nwin, C], BF16)
            h_sb = mp.tile([P, nwin, C], BF16)
            part_sb = mp.tile([P, nwin, HC + H], BF16)

            # ---------------- pass A + node phase ------------------------
            with (
                tc.tile_pool(name="ohA", bufs=2) as ohp,
                tc.tile_pool(name="wkA", bufs=2) as wp,
                tc.tile_pool(name="nodeA", bufs=2) as npl,
                tc.tile_pool(name="psA", bufs=2, space="PSUM") as ppa,
                tc.tile_pool(name="psT", bufs=1, space="PSUM") as ppt,
                tc.tile_pool(name="psM", bufs=1, space="PSUM") as ppm,
                tc.tile_pool(name="xedp", bufs=1) as xp,
            ):
                xed_sb = xp.tile([P, T, 8], BF16)
                nc.sync.dma_start(out=xed_sb[:], in_=xed_in[:])
                for w0, w1 in ga_groups:
                    t0 = int(wm_base[w0])
                    t1 = int(wm_base[w1])
                    gt = t1 - t0
                    oh_g = ohp.tile([P, GAT, P], BF16, tag="oh")
                    nc.sync.dma_start(
                        out=oh_g[:, 0:gt, :], in_=oh_in[:, t0:t1, :])

                    al = wp.tile([P, GAT, 5], F32, tag="al")
                    tm4 = wp.tile([P, GAT, 4], F32, tag="tm4")
                    tm6 = wp.tile([P, GAT, 6], F32, tag="tm6")
                    xs = xed_sb[:, t0:t1, :]
                    nc.vector.tensor_tensor(
                        out=al[:, 0:gt, 0:4],
                        in0=brd(cc_rep[:], [cc_rep[:].ap[0], [0, gt], [1, H]]),
                        in1=brd(xs, [xs.ap[0], [8, gt], [0, H]]),
                        op=OP.mult)
                    nc.gpsimd.tensor_tensor(
                        out=tm4[:, 0:gt, :],
                        in0=brd(cc_rep[:], [cc_rep[:].ap[0], [0, gt], [1, H]],
                                offset=cc_rep[:].offset + H),
                        in1=brd(xs, [xs.ap[0], [8, gt], [0, H]],
                                offset=xs.offset + 4),
                        op=OP.mult)
                    nc.vector.tensor_tensor(
                        out=al[:, 0:gt, 0:4], in0=al[:, 0:gt, 0:4],
                        in1=tm4[:, 0:gt, :], op=OP.add)
                    nc.vector.tensor_tensor(
                        out=al[:, 0:gt, 0:4], in0=al[:, 0:gt, 0:4],
                        in1=alE1[:, t0:t1, :], op=OP.add)
                    nc.gpsimd.tensor_tensor(
                        out=tm6[:, 0:gt, :],
                        in0=brd(xs, [xs.ap[0], [8, gt], [4, 2], [1, 3]],
                                offset=xs.offset + 1),
                        in1=brd(cds_rep[:], [cds_rep[:].ap[0], [0, gt], [3, 2], [1, 3]]),
                        op=OP.mult)
                    nc.vector.reduce_sum(
                        out=al[:, 0:gt, 4:5],
                        in_=brd(tm6[:], [tm6[:].ap[0], [6, gt], [1, 6]]),
                        axis=mybir.AxisListType.X)
                    alp = wp.tile([P, GAT, 5], F32, tag="alp")
                    nc.scalar.activation(alp[:, 0:gt, :], al[:, 0:gt, :],
                                         AF.Prelu, alpha=NEG)
                    rhsA = wp.tile([P, GAT, 12], BF16, tag="rhsA")
                    nc.scalar.activation(rhsA[:, 0:gt, 0:5], alp[:, 0:gt, :], AF.Exp)
                    nc.vector.tensor_tensor(
                        out=rhsA[:, 0:gt, 5:9],
                        in0=rhsA[:, 0:gt, 0:4],
                        in1=brd(xs, [xs.ap[0], [8, gt], [0, 4]]),
                        op=OP.mult)
                    nc.vector.tensor_tensor(
                        out=rhsA[:, 0:gt, 9:12],
                        in0=brd(xs, [xs.ap[0], [8, gt], [1, 3]], offset=xs.offset + 1),
                        in1=brd(rhsA[:], [rhsA[:].ap[0], [12, gt], [0, 3]],
                                offset=rhsA[:].offset + 4),
                        op=OP.mult)

                    for w in range(w0, w1):
                        toff = int(wm_base[w]) - t0
                        nt = int(tpw[w, 0] + tpw[w, 1])
                        psA = ppa.tile([P, 12], F32, tag="psA", space="PSUM")
                        for j in range(nt):
                            nc.tensor.matmul(
                                out=psA[:], lhsT=oh_g[:, toff + j, :],
                                rhs=rhsA[:, toff + j, :],
                                start=(j == 0), stop=(j == nt - 1))
                        den = wp.tile([P, 5], F32, tag="den")
                        nc.vector.tensor_scalar(
                            out=den[:], in0=psA[:, 0:5], scalar1=EPS, scalar2=None,
                            op0=OP.add)
                        nc.vector.reciprocal(out=den[:], in_=den[:])
                        nc.vector.tensor_tensor(
                            out=rA[:, w, 0:4], in0=psA[:, 5:9], in1=den[:, 0:4],
                            op=OP.mult)
                        nc.vector.tensor_tensor(
                            out=rA[:, w, 4:7], in0=psA[:, 9:12],
                            in1=den[:, 4:5].to_broadcast([P, 3]), op=OP.mult)

                        h1 = npl.tile([P, HC], F32, tag="h1")
                        nc.vector.tensor_tensor(
                            out=brd(h1[:], [h1[:].ap[0], [C, H], [1, C]]),
                            in0=brd(w1_rep[:], [w1_rep[:].ap[0], [C, H], [1, C]]),
                            in1=brd(rA[:], [rA[:].ap[0], [1, H], [0, C]],
                                    offset=rA[:].offset + w * 2 * H),
                            op=OP.mult)
                        nc.vector.tensor_tensor(
                            out=h1[:], in0=h1[:], in1=bs1_rep[:], op=OP.add)
                        rel = npl.tile([P, HC], F32, tag="rel")
                        nc.scalar.activation(rel[:], h1[:], AF.Relu)
                        nc.vector.tensor_tensor(
                            out=h1[:], in0=h1[:], in1=rel[:], op=OP.subtract)
                        nc.scalar.activation(h1[:], h1[:], AF.Exp)
                        h1e = npl.tile([P, HC], BF16, tag="h1e")
                        nc.vector.scalar_tensor_tensor(
                            out=h1e[:], in0=h1[:], scalar=-1.0, in1=rel[:],
                            op0=OP.add, op1=OP.add)
                        h1t = npl.tile([P, 2, P], BF16, tag="h1t")
                        for ch in range(2):
                            pst = ppt.tile([P, P], BF16, tag="tr", space="PSUM",
                                           bufs=2)
                            nc.tensor.transpose(
                                out=pst[:], in_=h1e[:, ch * P:(ch + 1) * P],
                                identity=ident_bf[:])
                            nc.scalar.copy(out=h1t[:, ch, :], in_=pst[:])
                        ph2 = ppm.tile([P, HC + 2 * H], F32, tag="mm", space="PSUM")
                        for ch in range(2):
                            nc.tensor.matmul(
                                out=ph2[:], lhsT=h1t[:, ch, :], rhs=ws2a_sb[:, ch, :],
                                start=(ch == 0), stop=(ch == 1))
                        h2row = npl.tile([P, HR], BF16, tag="h2row")
                        nc.vector.memset(h2row[:, HC + H:HR], 0.0)
                        nc.scalar.copy(out=h2row[:, 0:HC + H], in_=ph2[:, 0:HC + H])
                        nc.vector.tensor_copy(
                            out=sd2_all[:, w, :], in_=ph2[:, HC + H:HC + 2 * H])
                        if w < cbw:
                            nc.sync.dma_start(
                                out=h2sl[0][w * P:(w + 1) * P, :], in_=h2row[:])
                        else:
                            lw = w - cbw
                            nc.sync.dma_start(
                                out=h2sl[1][lw * P:(lw + 1) * P, :], in_=h2row[:])
                        prd = ppt.tile([P, P], F32, tag="trf", space="PSUM")
                        nc.tensor.transpose(
                            out=prd[0:3, :], in_=rA[:, w, 4:7], identity=ident[:])
                        rdt = npl.tile([3, P], BF16, tag="rdt")
                        nc.vector.tensor_copy(out=rdt[:], in_=prd[0:3, :])
                        phd = ppm.tile([P, C], F32, tag="mmd", space="PSUM")
                        nc.tensor.matmul(
                            out=phd[:], lhsT=rdt[:], rhs=wd_sb[:], start=True,
                            stop=True)
                        nc.vector.tensor_tensor(
                            out=hd_sb[:, w, :], in0=phd[:], in1=bd_rep[:], op=OP.add)
                        if w == cbw - 1 or w == nwin - 1:
                            kc = 0 if w == cbw - 1 else 1
                            nc.gpsimd.collective_compute(
                                "AllGather", OP.bypass,
                                replica_groups=[list(range(W))],
                                ins=[h2sl[kc][:]], outs=[H2ext[kc][:]])

            # ---------------- pass B: two sweeps -------------------------
            with (
                tc.tile_pool(name="ohB", bufs=2) as ohp,
                tc.tile_pool(name="hgB", bufs=3) as hgp,
                tc.tile_pool(name="wkB", bufs=2) as wp,
                tc.tile_pool(name="psB", bufs=2, space="PSUM") as ppb,
                tc.tile_pool(name="psS", bufs=2, space="PSUM") as pps,
                tc.tile_pool(name="psP", bufs=1, space="PSUM") as ppp,
                tc.tile_pool(name="idxp", bufs=1) as ixp,
            ):
                idx_sb = ixp.tile([P, T * 8], I16)
                nc.sync.dma_start(out=idx_sb[:], in_=idx_in[:])
                pg = ppp.tile([gpc, C], F32, tag="pg", space="PSUM")
                for sweep in range(2):
                    sm_base = T0 if sweep else 0
                    for w0, w1 in gb_groups:
                        t0 = sm_base + int(sm_off[sweep][w0])
                        t1 = sm_base + int(sm_off[sweep][w1])
                        gt = t1 - t0
                        ohb_g = ohp.tile([P, GBT, 2 * P], BF16, tag="ohb")
                        nc.sync.dma_start(
                            out=ohb_g[:, 0:gt, :], in_=ohbS_in[:, t0:t1, :])
                        hg = hgp.tile([P, GBT, HR], BF16, tag="hg")
                        for q0 in range(0, gt, GCAP):
                            qn = min(GCAP, gt - q0)
                            ts = t0 + q0
                            nc.gpsimd.dma_gather(
                                out_ap=hg[:, q0:q0 + qn, :],
                                in_ap=H2ext[sweep][:],
                                idxs_ap=idx_sb[:, ts * 8:(ts + qn) * 8],
                                num_idxs=qn * P, num_idxs_reg=qn * P,
                                elem_size=HR)
                        # dst attn values broadcast per window
                        s2dg = pps.tile([P, GBT * H], F32, tag="s2d", space="PSUM")
                        for w in range(w0, w1):
                            toff = int(sm_off[sweep][w]) - int(sm_off[sweep][w0])
                            for j in range(int(tpw[w, sweep])):
                                jj = toff + j
                                nc.tensor.matmul(
                                    out=s2dg[:, jj * H:(jj + 1) * H],
                                    lhsT=ohb_g[:, jj, P:2 * P],
                                    rhs=sd2_all[:, w, :],
                                    start=True, stop=True)
                        al2 = wp.tile([P, GBT, H], F32, tag="al2")
                        nc.vector.tensor_tensor(
                            out=al2[:, 0:gt, :],
                            in0=hg[:, 0:gt, HC:HC + H],
                            in1=brd(s2dg[:], [s2dg[:].ap[0], [H, gt], [1, H]]),
                            op=OP.add)
                        nc.vector.tensor_tensor(
                            out=al2[:, 0:gt, :], in0=al2[:, 0:gt, :],
                            in1=alE2[:, t0:t1, :], op=OP.add)
                        al2p = wp.tile([P, GBT, H], F32, tag="al2p")
                        nc.scalar.activation(al2p[:, 0:gt, :], al2[:, 0:gt, :],
                                             AF.Prelu, alpha=NEG)
                        rhsB = wp.tile([P, GBT, HC + H], BF16, tag="rhsB")
                        nc.scalar.activation(
                            rhsB[:, 0:gt, HC:HC + H], al2p[:, 0:gt, :], AF.Exp)
                        nc.vector.tensor_tensor(
                            out=brd(rhsB[:], [rhsB[:].ap[0], [HC + H, gt], [C, 3], [1, C]]),
                            in0=brd(hg[:], [hg[:].ap[0], [HR, gt], [C, 3], [1, C]]),
                            in1=brd(rhsB[:], [rhsB[:].ap[0], [HC + H, gt], [1, 3], [0, C]],
                                    offset=rhsB[:].offset + HC),
                            op=OP.mult)
                        nc.gpsimd.tensor_tensor(
                            out=brd(rhsB[:], [rhsB[:].ap[0], [HC + H, gt], [1, C]],
                                    offset=rhsB[:].offset + 3 * C),
                            in0=brd(hg[:], [hg[:].ap[0], [HR, gt], [1, C]],
                                    offset=hg[:].offset + 3 * C),
                            in1=brd(rhsB[:], [rhsB[:].ap[0], [HC + H, gt], [0, C]],
                                    offset=rhsB[:].offset + HC + 3),
                            op=OP.mult)
                        for w in range(w0, w1):
                            toff = int(sm_off[sweep][w]) - int(sm_off[sweep][w0])
                            ntk = int(tpw[w, sweep])
                            psB = ppb.tile([P, HC + H], F32, tag="psB", space="PSUM")
                            for j in range(ntk):
                                nc.tensor.matmul(
                                    out=psB[:], lhsT=ohb_g[:, toff + j, 0:P],
                                    rhs=rhsB[:, toff + j, :],
                                    start=(j == 0), stop=(j == ntk - 1))
                            if sweep == 0:
                                nc.vector.tensor_copy(
                                    out=part_sb[:, w, :], in_=psB[:])
                            else:
                                tot = wp.tile([P, HC + H], F32, tag="tot")
                                nc.vector.tensor_tensor(
                                    out=tot[:], in0=psB[:], in1=part_sb[:, w, :],
                                    op=OP.add)
                                dn2 = wp.tile([P, H], F32, tag="dn2")
                                nc.vector.tensor_scalar(
                                    out=dn2[:], in0=tot[:, HC:HC + H], scalar1=EPS,
                                    scalar2=None, op0=OP.add)
                                nc.vector.reciprocal(out=dn2[:], in_=dn2[:])
                                agg = wp.tile([P, HC], F32, tag="agg")
                                nc.vector.tensor_tensor(
                                    out=brd(agg[:], [agg[:].ap[0], [C, H], [1, C]]),
                                    in0=brd(tot[:], [tot[:].ap[0], [C, H], [1, C]]),
                                    in1=brd(dn2[:], [dn2[:].ap[0], [1, H], [0, C]]),
                                    op=OP.mult)
                                hf = wp.tile([P, C], F32, tag="hf")
                                nc.vector.reduce_sum(
                                    out=hf[:],
                                    in_=brd(agg[:], [agg[:].ap[0], [1, C], [C, H]]),
                                    axis=mybir.AxisListType.X)
                                nc.vector.scalar_tensor_tensor(
                                    out=hf[:], in0=hf[:], scalar=0.25,
                                    in1=bs2_rep[:], op0=OP.mult, op1=OP.add)
                                nc.vector.tensor_tensor(
                                    out=h_sb[:, w, :], in0=hf[:],
                                    in1=hd_sb[:, w, :], op=OP.add)
                                nc.tensor.matmul(
                                    out=pg[:], lhsT=pm_all[:, w, :],
                                    rhs=h_sb[:, w, :],
                                    start=(w == 0), stop=(w == nwin - 1))

            # ---------------- value head ---------------------------------
            with (
                tc.tile_pool(name="wkV", bufs=2) as wp,
                tc.tile_pool(name="psV", bufs=2, space="PSUM") as ppv,
            ):
                g_sb = wp.tile([gpc, C], F32, tag="g_sb")
                nc.vector.tensor_copy(out=g_sb[:], in_=pg[:])
                pgt = ppv.tile([C, gpc], F32, tag="tr", space="PSUM")
                nc.tensor.transpose(
                    out=pgt[:], in_=g_sb[:], identity=ident[0:gpc, 0:gpc])
                gt_sb = wp.tile([C, gpc], F32, tag="gt_sb")
                nc.vector.tensor_copy(out=gt_sb[:], in_=pgt[:])
                pv1 = ppv.tile([gpc, C], F32, tag="mm", space="PSUM")
                nc.tensor.matmul(
                    out=pv1[:], lhsT=gt_sb[:], rhs=wv1_sb[:], start=True, stop=True)
                a_sb = wp.tile([gpc, C], F32, tag="a_sb")
                nc.vector.tensor_tensor(
                    out=a_sb[:], in0=pv1[:], in1=bv1_rep[0:gpc, :], op=OP.add)
                nc.vector.tensor_scalar(
                    out=a_sb[:], in0=a_sb[:], scalar1=0.0, scalar2=None, op0=OP.max)
                pat = ppv.tile([C, gpc], F32, tag="tr", space="PSUM")
                nc.tensor.transpose(
                    out=pat[:], in_=a_sb[:], identity=ident[0:gpc, 0:gpc])
                at_sb = wp.tile([C, gpc], F32, tag="at_sb")
                nc.vector.tensor_copy(out=at_sb[:], in_=pat[:])
                pv2 = ppv.tile([gpc, 1], F32, tag="mm2", space="PSUM")
                nc.tensor.matmul(
                    out=pv2[:], lhsT=at_sb[:], rhs=wv2_sb[:], start=True, stop=True)
                vres = wp.tile([gpc, 1], F32, tag="vres")
                nc.vector.tensor_tensor(
                    out=vres[:], in0=pv2[:], in1=bv2_rep[0:gpc, :], op=OP.add)
                nc.sync.dma_start(out=v_out[:], in_=vres[:])

    nc.compile()
    return nc


# ----------------------------------------------------------------------------
# in_maps assembly
# ----------------------------------------------------------------------------

def make_in_maps(plan, per_core_arrays, weights):
    w = {k: np.ascontiguousarray(v, np.float32) for k, v in weights.items()}
    shared = dict(
        ws1=w["Ws1"].reshape(1, HC),
        a1s=w["as_src1"].reshape(1, HC),
        a1d=w["as_dst1"].reshape(1, HC),
        we1=w["We1"].reshape(1, 2 * HC),
        ae1=w["ae1"].reshape(1, HC),
        bs1=w["bs1"].reshape(1, HC),
        ws2=w["Ws2"],
        a2s=w["as_src2"].reshape(1, HC),
        a2d=w["as_dst2"].reshape(1, HC),
        we2=w["We2"].reshape(1, 2 * HC),
        ae2=w["ae2"].reshape(1, HC),
        bs2=w["bs2"].reshape(1, C),
        wd=w["Wd"],
        wdf=w["Wd"].reshape(1, 3 * C),
        ads=w["ad_src"].reshape(1, C),
        add=w["ad_dst"].reshape(1, C),
        bd=w["bd"].reshape(1, C),
        wv1=w["Wv1"],
        bv1=w["bv1"].reshape(1, C),
        wv2=w["Wv2"],
        bv2=w["bv2"].reshape(1, 1),
    )
    in_maps = []
    for c in range(plan.W):
        m = dict(shared)
        m.update(per_core_arrays[c])
        in_maps.append(m)
    return in_maps


_CACHE = {}


def prepare(inputs):
    x = np.asarray(inputs["x"])
    edge_attr = np.asarray(inputs["edge_attr"])
    edge_index = np.asarray(inputs["edge_index"])
    batch = np.asarray(inputs["batch"])
    G = 64
    W = 8
    plan, pca = host_prep(x, edge_attr, edge_index, batch, G, W)
    key = (plan.R, plan.T, plan.cbw, tuple(plan.tpw.ravel()))
    if key not in _CACHE:
        _CACHE[key] = build_bass(plan)
    nc = _CACHE[key]
    weights = {k: inputs[k] for k in (
        "Ws1", "as_src1", "as_dst1", "We1", "ae1", "bs1",
        "Ws2", "as_src2", "as_dst2", "We2", "ae2", "bs2",
        "Wd", "ad_src", "ad_dst", "bd", "Wv1", "bv1", "Wv2", "bv2")}
    in_maps = make_in_maps(plan, pca, weights)
    return nc, in_maps, plan


def kernel(**inputs):
    nc, in_maps, plan = prepare(inputs)
    from concourse.bass_utils import run_bass_kernel_spmd
    res = run_bass_kernel_spmd(nc, in_maps, list(range(plan.W)))
    v = np.concatenate([res.results[c]["v"][:, 0] for c in range(plan.W)])
    return v.astype(np.float32)
